# revision 8
# baseline (speedup 1.0000x reference)
"""Trainium2 Bass kernel for nn_Block_10024453669245 (dense transformer block).

Strategy (8 NeuronCores):
  - Phase A: per-core LN1 stats on its 512 own tokens + tiny AllGather.
  - Phase B: QKV tensor-parallel over heads (2 heads/core). fp32r matmuls
    against host-transposed xT; LN1 folded in as a rank-1 correction at
    PSUM eviction (qkv = (raw - s_col*murstd_row)*rstd_row).
    Produces qT,kT [d,t] and v [t,d] in bf16, resident in SBUF.
  - Phase C: causal attention, no-max-sub softmax (values are small),
    S^T = kT'.qT per 128x512 tile, exp on ACT, causal masks on diagonal
    tiles, O^T = v'.P^T accumulated in PSUM, denominator via ones-matmul.
  - Phase D: AllToAll (4MB/core) converts head-sharding -> token-sharding.
  - Phase E: MLP token-sharded (512 tokens/core) in bf16: x1T = xT_own +
    attnT, LN2 computed directly (stats via ones-matmuls), both matmuls
    stream full w1/w2 in bf16, gelu via ACT Gelu_apprx_tanh, residual add,
    output written transposed [C, 512] per core; host reassembles.
"""
import sys, math

sys.path.insert(0, "/opt/trn_rl_repo")

import numpy as np
import ml_dtypes

import concourse.bass as bass
import concourse.tile as tile
from concourse import bacc, mybir
from concourse.bass_utils import run_bass_kernel_spmd

# ---------------- constants (hardcoded problem shape) ----------------
P = 128
B, T, C = 2, 2048, 2048
H, D = 16, 128
R = 8                 # cores
HL = H // R           # heads per core
TOK = B * T // R      # own tokens per core
CT = C // P           # 16 c-tiles
NT = T // 512         # 4 t-blocks per batch
M1 = 4 * C            # 8192
MT = M1 // P          # 64 m-tiles
MG = 16               # m-groups of 4 m-tiles (512 cols) for matmul1
EPS = 1e-5
SCALE = 1.0 / math.sqrt(D)

F32 = mybir.dt.float32
F32R = mybir.dt.float32r
BF16 = mybir.dt.bfloat16
AF = mybir.ActivationFunctionType
ALU = mybir.AluOpType

_CACHE = {}
DEBUG = False


def _pbc(t, n_free):
    """partition-broadcast AP over a 1-D (or [1,n]) dram tile view."""
    return bass.AP(tensor=t.tensor, offset=t.offset, ap=[[0, P], [1, n_free]])


def _build():
    nc = bacc.Bacc("TRN2", target_bir_lowering=False, debug=False, num_devices=R)

    # ---------------- I/O ----------------
    xT_d = nc.dram_tensor("xT", [B, C, T], F32, kind="ExternalInput")
    x_own_d = nc.dram_tensor("x_own", [TOK, C], F32, kind="ExternalInput")
    xT_own_d = nc.dram_tensor("xT_own", [C, TOK], F32, kind="ExternalInput")
    wq_d = nc.dram_tensor("wq", [C, HL * D], F32, kind="ExternalInput")
    wk_d = nc.dram_tensor("wk", [C, HL * D], F32, kind="ExternalInput")
    wv_d = nc.dram_tensor("wv", [C, HL * D], F32, kind="ExternalInput")
    sq_d = nc.dram_tensor("sq", [HL * D], F32, kind="ExternalInput")
    sk_d = nc.dram_tensor("sk", [HL * D], F32, kind="ExternalInput")
    sv_d = nc.dram_tensor("sv", [HL * D], F32, kind="ExternalInput")
    w1_d = nc.dram_tensor("w1", [C, M1], BF16, kind="ExternalInput")
    w2r_d = nc.dram_tensor("w2r", [CT, MT, P, P], BF16, kind="ExternalInput")
    lnw_d = nc.dram_tensor("lnw", [C], F32, kind="ExternalInput")
    masks_d = nc.dram_tensor("masks", [4, P, 512], BF16, kind="ExternalInput")
    out_d = nc.dram_tensor("outT", [C, TOK], F32, kind="ExternalOutput")
    if DEBUG:
        dbg_stats = nc.dram_tensor("dbg_stats", [R, 2, TOK], F32, kind="ExternalOutput")
        dbg_qT = nc.dram_tensor("dbg_qT", [HL, B, P, T], BF16, kind="ExternalOutput")
        dbg_kT = nc.dram_tensor("dbg_kT", [HL, B, P, T], BF16, kind="ExternalOutput")
        dbg_v = nc.dram_tensor("dbg_v", [B, P, T // P, HL * D], BF16, kind="ExternalOutput")
        dbg_a2a = nc.dram_tensor("dbg_a2a", [R, HL, P, 512], F32, kind="ExternalOutput")
        dbg_x1 = nc.dram_tensor("dbg_x1", [C, TOK], F32, kind="ExternalOutput")
        dbg_h2 = nc.dram_tensor("dbg_h2", [P, CT, TOK], BF16, kind="ExternalOutput")

    with tile.TileContext(nc) as tc:
        with tc.tile_pool(name="dram", bufs=1, space="DRAM") as dram, \
             tc.tile_pool(name="psum", bufs=8, space="PSUM") as psum, \
             tc.tile_pool(name="singles", bufs=1) as singles:

            # internal DRAM
            stats_loc = dram.tile([2, TOK], F32)
            stats_g = dram.tile([R, 2, TOK], F32)
            a2a_in = dram.tile([R, HL, P, 512], F32)
            a2a_out = dram.tile([R, HL, P, 512], F32)
            x1_spill = dram.tile([C, TOK], F32)
            mlp_stat_b = dram.tile([2, TOK], F32)

            def ps():
                return psum.tile([P, 512], F32, tag="ps", name="ps")

            # small constants
            eps_t = singles.tile([P, 1], F32)
            nc.vector.memset(eps_t, EPS)
            ones_bf = singles.tile([P, 1], BF16)
            nc.vector.memset(ones_bf, 1.0)
            ones_f32 = singles.tile([P, 1], F32)
            nc.vector.memset(ones_f32, 1.0)

            # =========== Phase A: LN1 stats on own tokens ===========
            with tc.tile_pool(name="stA", bufs=3) as stA:
                for i in range(TOK // P):
                    xo = stA.tile([P, C], F32, tag="xo", name="xo")
                    nc.sync.dma_start(xo, x_own_d.ap()[i * P:(i + 1) * P, :])
                    xr = xo.rearrange("p (g s) -> p g s", s=512)
                    st = stA.tile([P, 4, 6], F32, tag="st", name="st")
                    for g in range(4):
                        nc.vector.bn_stats(out=st[:, g, :], in_=xr[:, g, :])
                    mv = stA.tile([P, 2], F32, tag="mv", name="mv")
                    nc.vector.bn_aggr(out=mv, in_=st)
                    rstd = stA.tile([P, 1], F32, tag="rstd", name="rstd")
                    nc.scalar.activation(rstd, mv[:, 1:2], AF.Sqrt, bias=eps_t)
                    nc.vector.reciprocal(rstd, rstd)
                    murstd = stA.tile([P, 1], F32, tag="murstd", name="murstd")
                    nc.vector.tensor_tensor(murstd, mv[:, 0:1], rstd, ALU.mult)
                    nc.sync.dma_start(
                        stats_loc[0, i * P:(i + 1) * P].rearrange("(p o) -> p o", o=1),
                        rstd)
                    nc.sync.dma_start(
                        stats_loc[1, i * P:(i + 1) * P].rearrange("(p o) -> p o", o=1),
                        murstd)
            nc.gpsimd.collective_compute(
                "AllGather", ALU.bypass,
                replica_groups=[list(range(R))],
                ins=[stats_loc.opt()], outs=[stats_g.opt()])

            # =========== Phase B+C pools ===========
            with tc.tile_pool(name="wqkv", bufs=1) as wpool, \
                 tc.tile_pool(name="qkvres", bufs=1) as qkvres, \
                 tc.tile_pool(name="xtp", bufs=3) as xtp, \
                 tc.tile_pool(name="reps", bufs=4) as reps, \
                 tc.tile_pool(name="tmps", bufs=4) as tmps, \
                 tc.tile_pool(name="attn", bufs=3) as attnp:

                wq_t = wpool.tile([P, CT, HL * D], F32R)
                nc.sync.dma_start(
                    wq_t, wq_d.ap().rearrange("(ko p) n -> p ko n", p=P).bitcast(F32R))
                wk_t = wpool.tile([P, CT, HL * D], F32R)
                nc.sync.dma_start(
                    wk_t, wk_d.ap().rearrange("(ko p) n -> p ko n", p=P).bitcast(F32R))
                wv_t = wpool.tile([P, CT, HL * D], F32R)
                nc.sync.dma_start(
                    wv_t, wv_d.ap().rearrange("(ko p) n -> p ko n", p=P).bitcast(F32R))
                sq_t = wpool.tile([P, HL], F32)
                nc.sync.dma_start(sq_t, sq_d.ap().rearrange("(hl d) -> d hl", d=P))
                sk_t = wpool.tile([P, HL], F32)
                nc.sync.dma_start(sk_t, sk_d.ap().rearrange("(hl d) -> d hl", d=P))
                sv_rep = wpool.tile([P, HL * D], F32)
                nc.sync.dma_start(sv_rep, bass.AP(
                    tensor=sv_d, offset=0, ap=[[0, P], [1, HL * D]]))
                masks_t = wpool.tile([P, 4, 512], BF16)
                nc.sync.dma_start(
                    masks_t, masks_d.ap().rearrange("m p t -> p m t"))

                # persistent qkv (bf16)
                qT = [[qkvres.tile([P, T], BF16, name=f"qT{h}{b}")
                       for b in range(B)] for h in range(HL)]
                kT = [[qkvres.tile([P, T], BF16, name=f"kT{h}{b}")
                       for b in range(B)] for h in range(HL)]
                vsb = [qkvres.tile([P, T // P, HL * D], BF16, name=f"v{b}")
                       for b in range(B)]

                # =========== Phase B: QKV ===========
                for b in range(B):
                    for tb in range(NT):
                        j = NT * b + tb
                        t0 = 512 * tb
                        # replicated row stats for this token block
                        rstd_rep = reps.tile([P, 512], F32, tag="rrep", name="rstd_rep")
                        nc.sync.dma_start(rstd_rep, _pbc(stats_g[j, 0, :], 512))
                        mur_rep = reps.tile([P, 512], F32, tag="mrep", name="mur_rep")
                        nc.sync.dma_start(mur_rep, _pbc(stats_g[j, 1, :], 512))

                        pq = [ps() for _ in range(HL)]
                        pk = [ps() for _ in range(HL)]
                        # one bank per 128-token v subtile: start=True clears the
                        # whole bank, so chains must not share a bank
                        pv = [ps() for _ in range(4)]
                        for ko in range(CT):
                            xt = xtp.tile([P, 512], F32R, tag="xt", name="xt")
                            nc.sync.dma_start(
                                xt,
                                xT_d.ap()[b, ko * P:(ko + 1) * P,
                                          t0:t0 + 512].bitcast(F32R))
                            st_flag = ko == 0
                            sp_flag = ko == CT - 1
                            for hl in range(HL):
                                nc.tensor.matmul(
                                    pq[hl], wq_t[:, ko, hl * D:(hl + 1) * D], xt,
                                    start=st_flag, stop=sp_flag)
                                nc.tensor.matmul(
                                    pk[hl], wk_t[:, ko, hl * D:(hl + 1) * D], xt,
                                    start=st_flag, stop=sp_flag)
                            for ss in range(4):
                                nc.tensor.matmul(
                                    pv[ss][:, 0:256],
                                    xt[:, ss * P:(ss + 1) * P], wv_t[:, ko, :],
                                    start=st_flag, stop=sp_flag)
                        # evict q/k: (raw - s*murstd)*rstd -> bf16
                        for hl in range(HL):
                            for (prm, s_col, dst) in ((pq[hl], sq_t[:, hl:hl + 1], qT[hl][b]),
                                                      (pk[hl], sk_t[:, hl:hl + 1], kT[hl][b])):
                                tmp = tmps.tile([P, 512], F32, tag="ev", name="ev")
                                nc.vector.tensor_scalar(tmp, mur_rep, s_col, None, ALU.mult)
                                nc.vector.tensor_tensor(tmp, prm, tmp, ALU.subtract)
                                nc.vector.tensor_tensor(
                                    dst[:, t0:t0 + 512], tmp, rstd_rep, ALU.mult)
                        # evict v per 128-token subtile
                        for ss in range(4):
                            si = tb * 4 + ss
                            rstd_c = tmps.tile([P, 1], F32, tag="rc", name="rc")
                            nc.sync.dma_start(
                                rstd_c,
                                stats_g[j, 0, ss * P:(ss + 1) * P]
                                .rearrange("(p o) -> p o", o=1))
                            mur_c = tmps.tile([P, 1], F32, tag="mc", name="mc")
                            nc.sync.dma_start(
                                mur_c,
                                stats_g[j, 1, ss * P:(ss + 1) * P]
                                .rearrange("(p o) -> p o", o=1))
                            tmp = tmps.tile([P, HL * D], F32, tag="evv", name="evv")
                            nc.vector.tensor_scalar(tmp, sv_rep, mur_c, None, ALU.mult)
                            nc.vector.tensor_tensor(
                                tmp, pv[ss][:, 0:256], tmp, ALU.subtract)
                            nc.vector.tensor_scalar(
                                vsb[b][:, si, :], tmp, rstd_c, None, ALU.mult)

                # =========== Phase C: attention ===========
                for b in range(B):
                    for hl in range(HL):
                        for tb in range(NT):
                            t0 = 512 * tb
                            n_s = 4 * (tb + 1)
                            pot = ps()
                            pden = ps()
                            for si in range(n_s):
                                pS = ps()
                                nc.tensor.matmul(
                                    pS, kT[hl][b][:, si * P:(si + 1) * P],
                                    qT[hl][b][:, t0:t0 + 512],
                                    start=True, stop=True)
                                pt = attnp.tile([P, 512], BF16, tag="pt", name="pt")
                                nc.scalar.activation(pt, pS, AF.Exp)
                                m = si - (n_s - 4)
                                if m >= 0:
                                    nc.vector.tensor_tensor(
                                        pt, pt, masks_t[:, m, :], ALU.mult)
                                nc.tensor.matmul(
                                    pot, vsb[b][:, si, hl * D:(hl + 1) * D], pt,
                                    start=(si == 0), stop=(si == n_s - 1))
                                nc.tensor.matmul(
                                    pden[0:1, :], ones_bf, pt,
                                    start=(si == 0), stop=(si == n_s - 1))
                            # normalize: OT / den
                            den_r = attnp.tile([1, 512], F32, tag="dr", name="den_r")
                            nc.vector.reciprocal(den_r, pden[0:1, :])
                            den_d = dram.tile([512], F32, tag="den_d", bufs=4,
                                              name="den_d")
                            nc.sync.dma_start(den_d.rearrange("(o t) -> o t", o=1), den_r)
                            den_rep = reps.tile([P, 512], F32, tag="denrep",
                                                name="den_rep")
                            nc.sync.dma_start(den_rep, _pbc(den_d, 512))
                            ot = attnp.tile([P, 512], F32, tag="ot", name="ot")
                            nc.vector.tensor_tensor(ot, pot, den_rep, ALU.mult)
                            nc.sync.dma_start(a2a_in[NT * b + tb, hl, :, :], ot)

                if DEBUG:
                    for hl in range(HL):
                        for b in range(B):
                            nc.sync.dma_start(dbg_qT.ap()[hl, b], qT[hl][b])
                            nc.sync.dma_start(dbg_kT.ap()[hl, b], kT[hl][b])
                    for b in range(B):
                        nc.sync.dma_start(dbg_v.ap()[b], vsb[b])
                    nc.sync.dma_start(dbg_a2a.ap(), a2a_in)
                    nc.sync.dma_start(dbg_stats.ap(), stats_g)

            # =========== Phase D: AllToAll ===========
            nc.gpsimd.collective_compute(
                "AllToAll", ALU.bypass,
                replica_groups=[list(range(R))],
                ins=[a2a_in.opt()], outs=[a2a_out.opt()])

            # =========== Phase E: MLP (token-sharded, bf16) ===========
            attnT_view = a2a_out.rearrange("r h d t -> (r h d) t")
            with tc.tile_pool(name="mlp_x1", bufs=3) as x1p, \
                 tc.tile_pool(name="mlp_sq", bufs=2) as sqp, \
                 tc.tile_pool(name="mlp_h2", bufs=1) as h2p, \
                 tc.tile_pool(name="mlp_g", bufs=1) as gp, \
                 tc.tile_pool(name="mlp_w1", bufs=24) as w1p, \
                 tc.tile_pool(name="mlp_w2", bufs=2) as w2p, \
                 tc.tile_pool(name="mlp_reps", bufs=1) as mreps, \
                 tc.tile_pool(name="mlp_out", bufs=2) as outp:

                lnw_t = singles.tile([P, CT], F32)
                nc.sync.dma_start(lnw_t, lnw_d.ap().rearrange("(ko p) -> p ko", p=P))

                # pass 1: build x1T tiles, stats, spill
                pmu = ps()
                psq = ps()
                for ct in range(CT):
                    xo = x1p.tile([P, TOK], F32, tag="xo2", name="xo2")
                    nc.sync.dma_start(xo, xT_own_d.ap()[ct * P:(ct + 1) * P, :])
                    at = x1p.tile([P, TOK], F32, tag="at", name="at")
                    nc.sync.dma_start(at, attnT_view[ct * P:(ct + 1) * P, :])
                    x1 = x1p.tile([P, TOK], F32, tag="x1", name="x1")
                    nc.vector.tensor_tensor(x1, xo, at, ALU.add)
                    nc.sync.dma_start(x1_spill[ct * P:(ct + 1) * P, :], x1)
                    sq2 = sqp.tile([P, TOK], F32, tag="sq2", name="sq2")
                    nc.vector.tensor_tensor(sq2, x1, x1, ALU.mult)
                    nc.tensor.matmul(pmu[0:1, :], ones_f32, x1,
                                     start=(ct == 0), stop=(ct == CT - 1))
                    nc.tensor.matmul(psq[0:1, :], ones_f32, sq2,
                                     start=(ct == 0), stop=(ct == CT - 1))
                # finalize stats: mu = pmu/C ; var = psq/C - mu^2
                mu2 = singles.tile([1, TOK], F32)
                nc.vector.tensor_scalar(mu2, pmu[0:1, :], 1.0 / C, None, ALU.mult)
                var2 = singles.tile([1, TOK], F32)
                nc.vector.tensor_scalar(var2, psq[0:1, :], 1.0 / C, None, ALU.mult)
                musq = singles.tile([1, TOK], F32)
                nc.vector.tensor_tensor(musq, mu2, mu2, ALU.mult)
                nc.vector.tensor_tensor(var2, var2, musq, ALU.subtract)
                rstd2 = singles.tile([1, TOK], F32)
                nc.scalar.activation(rstd2, var2, AF.Sqrt, bias=eps_t[0:1])
                nc.vector.reciprocal(rstd2, rstd2)
                nc.sync.dma_start(
                    mlp_stat_b[0, :].rearrange("(o t) -> o t", o=1), mu2)
                nc.sync.dma_start(
                    mlp_stat_b[1, :].rearrange("(o t) -> o t", o=1), rstd2)
                mu2_rep = mreps.tile([P, TOK], F32, name="mu2_rep")
                nc.sync.dma_start(mu2_rep, _pbc(mlp_stat_b[0, :], TOK))
                rstd2_rep = mreps.tile([P, TOK], F32, name="rstd2_rep")
                nc.sync.dma_start(rstd2_rep, _pbc(mlp_stat_b[1, :], TOK))

                # pass 2: h2T (bf16) from spilled x1
                h2 = h2p.tile([P, CT, TOK], BF16)
                for ct in range(CT):
                    x1r = x1p.tile([P, TOK], F32, tag="x1r", name="x1r")
                    nc.sync.dma_start(x1r, x1_spill[ct * P:(ct + 1) * P, :])
                    t1 = sqp.tile([P, TOK], F32, tag="t1", name="t1")
                    nc.vector.tensor_tensor(t1, x1r, mu2_rep, ALU.subtract)
                    nc.vector.tensor_tensor(t1, t1, rstd2_rep, ALU.mult)
                    nc.vector.tensor_scalar(
                        h2[:, ct, :], t1, lnw_t[:, ct:ct + 1], None, ALU.mult)

                if DEBUG:
                    nc.sync.dma_start(dbg_x1.ap(), x1_spill)
                    nc.sync.dma_start(dbg_h2.ap(), h2)

                # matmul1 + gelu -> gT
                gT = gp.tile([P, MT, TOK], BF16)
                for mg in range(MG):
                    wts = []
                    for ct in range(CT):
                        w1t = w1p.tile([P, 512], BF16, tag="w1t", name="w1t")
                        nc.sync.dma_start(
                            w1t, w1_d.ap()[ct * P:(ct + 1) * P,
                                           mg * 512:(mg + 1) * 512])
                        wts.append(w1t)
                    pg = [ps() for _ in range(4)]
                    for ct in range(CT):
                        for ml in range(4):
                            nc.tensor.matmul(
                                pg[ml], wts[ct][:, ml * P:(ml + 1) * P], h2[:, ct, :],
                                start=(ct == 0), stop=(ct == CT - 1))
                    for ml in range(4):
                        nc.scalar.activation(
                            gT[:, mg * 4 + ml, :], pg[ml], AF.Gelu_apprx_tanh)

                # matmul2 + residual -> outT
                for co in range(CT):
                    w2t = w2p.tile([P, MT, P], BF16, tag="w2t", name="w2t")
                    nc.sync.dma_start(
                        w2t, w2r_d.ap()[co].rearrange("mo p c -> p mo c"))
                    po = ps()
                    for mt in range(MT):
                        nc.tensor.matmul(po, w2t[:, mt, :], gT[:, mt, :],
                                         start=(mt == 0), stop=(mt == MT - 1))
                    x1r = x1p.tile([P, TOK], F32, tag="x1o", name="x1o")
                    nc.sync.dma_start(x1r, x1_spill[co * P:(co + 1) * P, :])
                    ot2 = outp.tile([P, TOK], F32, tag="ot2", name="ot2")
                    nc.vector.tensor_tensor(ot2, po, x1r, ALU.add)
                    nc.sync.dma_start(out_d.ap()[co * P:(co + 1) * P, :], ot2)

    nc.compile()
    return nc


def _host_prep(x, w_qkv, w1, w2, ln_w):
    x = np.asarray(x, dtype=np.float32)
    w_qkv = np.asarray(w_qkv, dtype=np.float32)
    w1 = np.asarray(w1, dtype=np.float32)
    w2 = np.asarray(w2, dtype=np.float32)
    ln_w = np.asarray(ln_w, dtype=np.float32)

    xT = np.ascontiguousarray(x.transpose(0, 2, 1))            # [B, C, T]
    x_flat = x.reshape(B * T, C)

    Wq = (ln_w[:, None] * w_qkv[:, 0 * C:1 * C]) * SCALE
    Wk = ln_w[:, None] * w_qkv[:, 1 * C:2 * C]
    Wv = ln_w[:, None] * w_qkv[:, 2 * C:3 * C]
    sq_full = Wq.sum(0, dtype=np.float64).astype(np.float32)
    sk_full = Wk.sum(0, dtype=np.float64).astype(np.float32)
    sv_full = Wv.sum(0, dtype=np.float64).astype(np.float32)

    w1_bf = w1.astype(ml_dtypes.bfloat16)
    # w2 reordered: [CT, MT, P(m), P(c)]
    w2r = np.ascontiguousarray(
        w2.reshape(MT, P, CT, P).transpose(2, 0, 1, 3)).astype(ml_dtypes.bfloat16)

    masks = np.zeros((4, P, 512), np.float32)
    for m in range(4):
        s_idx = np.arange(P)[:, None] + P * m
        t_idx = np.arange(512)[None, :]
        masks[m] = (t_idx >= s_idx).astype(np.float32)
    masks = masks.astype(ml_dtypes.bfloat16)

    in_maps = []
    for r in range(R):
        cs = slice(256 * r, 256 * (r + 1))
        b_own, tb_own = r // NT, r % NT
        in_maps.append({
            "xT": xT,
            "x_own": np.ascontiguousarray(x_flat[TOK * r: TOK * (r + 1)]),
            "xT_own": np.ascontiguousarray(
                xT[b_own][:, 512 * tb_own: 512 * (tb_own + 1)]),
            "wq": np.ascontiguousarray(Wq[:, cs]),
            "wk": np.ascontiguousarray(Wk[:, cs]),
            "wv": np.ascontiguousarray(Wv[:, cs]),
            "sq": np.ascontiguousarray(sq_full[cs]),
            "sk": np.ascontiguousarray(sk_full[cs]),
            "sv": np.ascontiguousarray(sv_full[cs]),
            "w1": w1_bf,
            "w2r": w2r,
            "lnw": ln_w,
            "masks": masks,
        })
    return in_maps


def get_nc():
    if "nc" not in _CACHE:
        _CACHE["nc"] = _build()
    return _CACHE["nc"]


def run(in_maps, **kw):
    nc = get_nc()
    return run_bass_kernel_spmd(nc, in_maps, core_ids=list(range(R)), **kw)


def kernel(x, w_qkv, w1, w2, ln_w, **kw_unused):
    in_maps = _host_prep(x, w_qkv, w1, w2, ln_w)
    res = run(in_maps)
    out_flat = np.empty((B * T, C), np.float32)
    for r in range(R):
        out_flat[TOK * r: TOK * (r + 1)] = res.results[r]["outT"].T
    return out_flat.reshape(B, T, C)


# revision 12
# speedup vs baseline: 1.0823x; 1.0823x over previous
"""Trainium2 Bass kernel for nn_Block_10024453669245 (dense transformer block).

Strategy (8 NeuronCores):
  - warmup: dummy 32B AllGather prepays collective-communicator init.
  - Phase A: per-core LN1 stats on its 512 own tokens + tiny AllGather.
  - Phase B: QKV tensor-parallel over heads (2 heads/core). fp32r matmuls
    against host-transposed xT. LN1 is folded in: the rank-1 term
    (-colsum x murstd) is added via a K=1 fp32r matmul inside the same
    PSUM accumulation group; eviction is a single DVE mult by rstd.
    Produces qT,kT [d,t] and v [t,d] in bf16, resident in SBUF.
  - Phase C: causal attention head-major, no-max-sub softmax, S^T tiles,
    exp on ACT, causal masks on diagonal tiles, O^T and denominator
    accumulated on PE. Per-head AllToAll (2MB) fires as soon as that
    head's outputs are done, overlapping the other head's attention.
  - Phase E: MLP token-sharded (512 tokens/core) in bf16. ln2's weight is
    folded into w1 (host), the mean term via K=1 matmul fold, rstd2 at
    PSUM eviction. gelu = ACT Gelu_apprx_tanh. Residual from f32 spill.
    Output written transposed [C, 512] per core; host reassembles.
  DMAs are spread over sync/gpsimd/vector queues to avoid serializing.
"""
import sys, math

sys.path.insert(0, "/opt/trn_rl_repo")

import numpy as np
import ml_dtypes

import concourse.bass as bass
import concourse.tile as tile
from concourse import bacc, mybir
from concourse.bass_utils import run_bass_kernel_spmd

# ---------------- constants (hardcoded problem shape) ----------------
P = 128
B, T, C = 2, 2048, 2048
H, D = 16, 128
R = 8                 # cores
HL = H // R           # heads per core
TOK = B * T // R      # own tokens per core
CT = C // P           # 16 c-tiles
NT = T // 512         # 4 t-blocks per batch
M1 = 4 * C            # 8192
MT = M1 // P          # 64 m-tiles
MG = 16               # m-groups of 4 m-tiles (512 cols) for matmul1
EPS = 1e-5
SCALE = 1.0 / math.sqrt(D)

F32 = mybir.dt.float32
F32R = mybir.dt.float32r
BF16 = mybir.dt.bfloat16
AF = mybir.ActivationFunctionType
ALU = mybir.AluOpType

_CACHE = {}
DEBUG = False


def _pbc(t, n_free):
    """partition-broadcast AP over a 1-D dram tile view."""
    return bass.AP(tensor=t.tensor, offset=t.offset, ap=[[0, P], [1, n_free]])


def _row(ap1d):
    return ap1d.rearrange("(o t) -> o t", o=1)


def _build():
    nc = bacc.Bacc("TRN2", target_bir_lowering=False, debug=False, num_devices=R)

    # ---------------- I/O ----------------
    xT_d = nc.dram_tensor("xT", [B, C, T], F32, kind="ExternalInput")
    x_own_d = nc.dram_tensor("x_own", [TOK, C], F32, kind="ExternalInput")
    xT_own_d = nc.dram_tensor("xT_own", [C, TOK], F32, kind="ExternalInput")
    wq_d = nc.dram_tensor("wq", [C, HL * D], F32, kind="ExternalInput")
    wk_d = nc.dram_tensor("wk", [C, HL * D], F32, kind="ExternalInput")
    wv_d = nc.dram_tensor("wv", [C, HL * D], F32, kind="ExternalInput")
    nsq_d = nc.dram_tensor("nsq", [HL * D], F32, kind="ExternalInput")
    nsk_d = nc.dram_tensor("nsk", [HL * D], F32, kind="ExternalInput")
    nsv_d = nc.dram_tensor("nsv", [HL * D], F32, kind="ExternalInput")
    w1_d = nc.dram_tensor("w1", [C, M1], BF16, kind="ExternalInput")
    ns1_d = nc.dram_tensor("ns1", [M1], F32, kind="ExternalInput")
    w2r_d = nc.dram_tensor("w2r", [CT, MT, P, P], BF16, kind="ExternalInput")
    masks_d = nc.dram_tensor("masks", [4, P, 512], BF16, kind="ExternalInput")
    out_d = nc.dram_tensor("outT", [C, TOK], F32, kind="ExternalOutput")

    with tile.TileContext(nc) as tc:
        with tc.tile_pool(name="dram", bufs=1, space="DRAM") as dram, \
             tc.tile_pool(name="psum", bufs=8, space="PSUM") as psum, \
             tc.tile_pool(name="singles", bufs=1) as singles:

            # internal DRAM
            warm_in = dram.tile([8], F32)
            warm_out = dram.tile([R, 8], F32)
            stats_loc = dram.tile([2, TOK], F32)
            stats_g = dram.tile([R, 2, TOK], F32)
            a2a_in = [dram.tile([R, P, 512], F32, name=f"a2a_in{h}")
                      for h in range(HL)]
            a2a_out = [dram.tile([R, P, 512], F32, name=f"a2a_out{h}")
                       for h in range(HL)]
            x1_spill = dram.tile([C, TOK], F32)
            mlp_stat_b = dram.tile([2, TOK], F32)

            def ps():
                return psum.tile([P, 512], F32, tag="ps", name="ps")

            # warmup collective: pays communicator init while phase A runs
            nc.gpsimd.collective_compute(
                "AllGather", ALU.bypass, replica_groups=[list(range(R))],
                ins=[warm_in.opt()], outs=[warm_out.opt()])

            # small constants
            eps_t = singles.tile([P, 1], F32)
            nc.vector.memset(eps_t, EPS)
            ones_bf = singles.tile([P, 1], BF16)
            nc.vector.memset(ones_bf, 1.0)
            ones_f32 = singles.tile([P, 1], F32)
            nc.vector.memset(ones_f32, 1.0)

            # =========== Phase A: LN1 stats on own tokens ===========
            with tc.tile_pool(name="stA", bufs=3) as stA:
                for i in range(TOK // P):
                    xo = stA.tile([P, C], F32, tag="xo", name="xo")
                    nc.gpsimd.dma_start(xo, x_own_d.ap()[i * P:(i + 1) * P, :])
                    xr = xo.rearrange("p (g s) -> p g s", s=512)
                    st = stA.tile([P, 4, 6], F32, tag="st", name="st")
                    for g in range(4):
                        nc.vector.bn_stats(out=st[:, g, :], in_=xr[:, g, :])
                    mv = stA.tile([P, 2], F32, tag="mv", name="mv")
                    nc.vector.bn_aggr(out=mv, in_=st)
                    rstd = stA.tile([P, 1], F32, tag="rstd", name="rstd")
                    nc.scalar.activation(rstd, mv[:, 1:2], AF.Sqrt, bias=eps_t)
                    nc.vector.reciprocal(rstd, rstd)
                    murstd = stA.tile([P, 1], F32, tag="murstd", name="murstd")
                    nc.vector.tensor_tensor(murstd, mv[:, 0:1], rstd, ALU.mult)
                    nc.gpsimd.dma_start(
                        stats_loc[0, i * P:(i + 1) * P].rearrange("(p o) -> p o", o=1),
                        rstd)
                    nc.gpsimd.dma_start(
                        stats_loc[1, i * P:(i + 1) * P].rearrange("(p o) -> p o", o=1),
                        murstd)
            nc.gpsimd.collective_compute(
                "AllGather", ALU.bypass,
                replica_groups=[list(range(R))],
                ins=[stats_loc.opt()], outs=[stats_g.opt()])

            # =========== Phase B+C pools ===========
            with tc.tile_pool(name="wqkv", bufs=1) as wpool, \
                 tc.tile_pool(name="qkvres", bufs=1) as qkvres, \
                 tc.tile_pool(name="xtp", bufs=4) as xtp, \
                 tc.tile_pool(name="reps", bufs=4) as reps, \
                 tc.tile_pool(name="tmps", bufs=4) as tmps, \
                 tc.tile_pool(name="attn", bufs=3) as attnp:

                wq_t = wpool.tile([P, CT, HL * D], F32R)
                nc.sync.dma_start(
                    wq_t, wq_d.ap().rearrange("(ko p) n -> p ko n", p=P).bitcast(F32R))
                wk_t = wpool.tile([P, CT, HL * D], F32R)
                nc.sync.dma_start(
                    wk_t, wk_d.ap().rearrange("(ko p) n -> p ko n", p=P).bitcast(F32R))
                wv_t = wpool.tile([P, CT, HL * D], F32R)
                nc.sync.dma_start(
                    wv_t, wv_d.ap().rearrange("(ko p) n -> p ko n", p=P).bitcast(F32R))
                nsq_t = wpool.tile([1, HL * D], F32R)
                nc.gpsimd.dma_start(nsq_t, _row(nsq_d.ap()).bitcast(F32R))
                nsk_t = wpool.tile([1, HL * D], F32R)
                nc.gpsimd.dma_start(nsk_t, _row(nsk_d.ap()).bitcast(F32R))
                nsv_t = wpool.tile([1, HL * D], F32R)
                nc.gpsimd.dma_start(nsv_t, _row(nsv_d.ap()).bitcast(F32R))
                masks_t = wpool.tile([P, 4, 512], BF16)
                nc.sync.dma_start(
                    masks_t, masks_d.ap().rearrange("m p t -> p m t"))

                # persistent qkv (bf16)
                qT = [[qkvres.tile([P, T], BF16, name=f"qT{h}{b}")
                       for b in range(B)] for h in range(HL)]
                kT = [[qkvres.tile([P, T], BF16, name=f"kT{h}{b}")
                       for b in range(B)] for h in range(HL)]
                vsb = [qkvres.tile([P, T // P, HL * D], BF16, name=f"v{b}")
                       for b in range(B)]

                # =========== Phase B: QKV ===========
                for b in range(B):
                    for tb in range(NT):
                        j = NT * b + tb
                        t0 = 512 * tb
                        murow = reps.tile([1, 512], F32R, tag="murow", name="murow")
                        nc.gpsimd.dma_start(
                            murow, _row(stats_g[j, 1, :]).bitcast(F32R))
                        rstd_rep = reps.tile([P, 512], F32, tag="rrep", name="rstd_rep")
                        nc.gpsimd.dma_start(rstd_rep, _pbc(stats_g[j, 0, :], 512))

                        pq = [ps() for _ in range(HL)]
                        pk = [ps() for _ in range(HL)]
                        # one bank per 128-token v subtile (start=True clears
                        # the whole bank, chains must not share one)
                        pv = [ps() for _ in range(4)]
                        for ko in range(CT):
                            xt = xtp.tile([P, 512], F32R, tag="xt", name="xt")
                            nc.sync.dma_start(
                                xt,
                                xT_d.ap()[b, ko * P:(ko + 1) * P,
                                          t0:t0 + 512].bitcast(F32R))
                            st_flag = ko == 0
                            for hl in range(HL):
                                nc.tensor.matmul(
                                    pq[hl], wq_t[:, ko, hl * D:(hl + 1) * D], xt,
                                    start=st_flag, stop=False)
                                nc.tensor.matmul(
                                    pk[hl], wk_t[:, ko, hl * D:(hl + 1) * D], xt,
                                    start=st_flag, stop=False)
                            for ss in range(4):
                                nc.tensor.matmul(
                                    pv[ss][:, 0:256],
                                    xt[:, ss * P:(ss + 1) * P], wv_t[:, ko, :],
                                    start=st_flag, stop=False)
                        # rank-1 LN fold: += (-colsum) x murstd  (K=1 matmul)
                        for hl in range(HL):
                            nc.tensor.matmul(
                                pq[hl], nsq_t[0:1, hl * D:(hl + 1) * D], murow,
                                start=False, stop=True)
                            nc.tensor.matmul(
                                pk[hl], nsk_t[0:1, hl * D:(hl + 1) * D], murow,
                                start=False, stop=True)
                        for ss in range(4):
                            nc.tensor.matmul(
                                pv[ss][:, 0:256],
                                murow[0:1, ss * P:(ss + 1) * P], nsv_t,
                                start=False, stop=True)
                        # evictions: single mult by rstd
                        for hl in range(HL):
                            nc.vector.tensor_tensor(
                                qT[hl][b][:, t0:t0 + 512], pq[hl], rstd_rep, ALU.mult)
                            nc.vector.tensor_tensor(
                                kT[hl][b][:, t0:t0 + 512], pk[hl], rstd_rep, ALU.mult)
                        for ss in range(4):
                            si = tb * 4 + ss
                            rstd_c = tmps.tile([P, 1], F32, tag="rc", name="rc")
                            nc.gpsimd.dma_start(
                                rstd_c,
                                stats_g[j, 0, ss * P:(ss + 1) * P]
                                .rearrange("(p o) -> p o", o=1))
                            nc.vector.tensor_scalar(
                                vsb[b][:, si, :], pv[ss][:, 0:256], rstd_c, None,
                                ALU.mult)

                # =========== Phase C: attention (head-major) ===========
                for hl in range(HL):
                    for b in range(B):
                        for tb in range(NT):
                            t0 = 512 * tb
                            n_s = 4 * (tb + 1)
                            pot = ps()
                            pden = ps()
                            for si in range(n_s):
                                pS = ps()
                                nc.tensor.matmul(
                                    pS, kT[hl][b][:, si * P:(si + 1) * P],
                                    qT[hl][b][:, t0:t0 + 512],
                                    start=True, stop=True)
                                pt = attnp.tile([P, 512], BF16, tag="pt", name="pt")
                                nc.scalar.activation(pt, pS, AF.Exp)
                                m = si - (n_s - 4)
                                if m >= 0:
                                    nc.vector.tensor_tensor(
                                        pt, pt, masks_t[:, m, :], ALU.mult)
                                nc.tensor.matmul(
                                    pot, vsb[b][:, si, hl * D:(hl + 1) * D], pt,
                                    start=(si == 0), stop=(si == n_s - 1))
                                nc.tensor.matmul(
                                    pden[0:1, :], ones_bf, pt,
                                    start=(si == 0), stop=(si == n_s - 1))
                            # normalize: OT / den
                            den_r = attnp.tile([1, 512], F32, tag="dr", name="den_r")
                            nc.vector.reciprocal(den_r, pden[0:1, :])
                            den_d = dram.tile([512], F32, tag="den_d", bufs=4,
                                              name="den_d")
                            nc.gpsimd.dma_start(_row(den_d), den_r)
                            den_rep = reps.tile([P, 512], F32, tag="denrep",
                                                name="den_rep")
                            nc.gpsimd.dma_start(den_rep, _pbc(den_d, 512))
                            ot = attnp.tile([P, 512], F32, tag="ot", name="ot")
                            nc.vector.tensor_tensor(ot, pot, den_rep, ALU.mult)
                            nc.gpsimd.dma_start(a2a_in[hl][NT * b + tb, :, :], ot)
                    # per-head AllToAll fires as soon as head hl is done
                    nc.gpsimd.collective_compute(
                        "AllToAll", ALU.bypass,
                        replica_groups=[list(range(R))],
                        ins=[a2a_in[hl].opt()], outs=[a2a_out[hl].opt()])

            # =========== Phase E: MLP (token-sharded, bf16) ===========
            with tc.tile_pool(name="mlp_x1", bufs=3) as x1p, \
                 tc.tile_pool(name="mlp_sq", bufs=2) as sqp, \
                 tc.tile_pool(name="mlp_x1bf", bufs=1) as x1bfp, \
                 tc.tile_pool(name="mlp_g", bufs=1) as gp, \
                 tc.tile_pool(name="mlp_w1", bufs=22) as w1p, \
                 tc.tile_pool(name="mlp_w2", bufs=2) as w2p, \
                 tc.tile_pool(name="mlp_z", bufs=4) as zp, \
                 tc.tile_pool(name="mlp_out", bufs=2) as outp:

                x1bf = x1bfp.tile([P, CT, TOK], BF16)
                # pass 1: build x1 tiles (evens first: only need a2a head 0),
                # stats matmuls, bf16 copy, f32 spill
                pmu = ps()
                psq = ps()
                order = [2 * i for i in range(CT // 2)] + \
                        [2 * i + 1 for i in range(CT // 2)]
                for idx, ct in enumerate(order):
                    xo = x1p.tile([P, TOK], F32, tag="xo2", name="xo2")
                    nc.gpsimd.dma_start(xo, xT_own_d.ap()[ct * P:(ct + 1) * P, :])
                    at = x1p.tile([P, TOK], F32, tag="at", name="at")
                    nc.gpsimd.dma_start(at, a2a_out[ct % 2][ct // 2])
                    x1 = x1p.tile([P, TOK], F32, tag="x1", name="x1")
                    nc.vector.tensor_tensor(x1, xo, at, ALU.add)
                    nc.gpsimd.dma_start(x1_spill[ct * P:(ct + 1) * P, :], x1)
                    nc.vector.tensor_copy(x1bf[:, ct, :], x1)
                    sq2 = sqp.tile([P, TOK], F32, tag="sq2", name="sq2")
                    nc.vector.tensor_tensor(sq2, x1, x1, ALU.mult)
                    nc.tensor.matmul(pmu[0:1, :], ones_f32, x1,
                                     start=(idx == 0), stop=(idx == CT - 1))
                    nc.tensor.matmul(psq[0:1, :], ones_f32, sq2,
                                     start=(idx == 0), stop=(idx == CT - 1))
                # finalize stats: mu = pmu/C ; var = psq/C - mu^2
                mu2 = singles.tile([1, TOK], F32)
                nc.vector.tensor_scalar(mu2, pmu[0:1, :], 1.0 / C, None, ALU.mult)
                var2 = singles.tile([1, TOK], F32)
                nc.vector.tensor_scalar(var2, psq[0:1, :], 1.0 / C, None, ALU.mult)
                musq = singles.tile([1, TOK], F32)
                nc.vector.tensor_tensor(musq, mu2, mu2, ALU.mult)
                nc.vector.tensor_tensor(var2, var2, musq, ALU.subtract)
                rstd2 = singles.tile([1, TOK], F32)
                nc.scalar.activation(rstd2, var2, AF.Sqrt, bias=eps_t[0:1])
                nc.vector.reciprocal(rstd2, rstd2)
                nc.gpsimd.dma_start(_row(mlp_stat_b[0, :]), mu2)
                nc.gpsimd.dma_start(_row(mlp_stat_b[1, :]), rstd2)
                murow2 = singles.tile([1, TOK], F32R)
                nc.gpsimd.dma_start(murow2, _row(mlp_stat_b[0, :]).bitcast(F32R))
                rstd2_rep = singles.tile([P, TOK], F32)
                nc.gpsimd.dma_start(rstd2_rep, _pbc(mlp_stat_b[1, :], TOK))

                # matmul1 (+ mean fold) -> *rstd2 -> gelu -> gT
                gT = gp.tile([P, MT, TOK], BF16)
                for mg in range(MG):
                    ns1g = zp.tile([1, 512], F32R, tag="ns1g", name="ns1g")
                    nc.gpsimd.dma_start(
                        ns1g, _row(ns1_d.ap()[mg * 512:(mg + 1) * 512]).bitcast(F32R))
                    wts = []
                    for ct in range(CT):
                        w1t = w1p.tile([P, 512], BF16, tag="w1t", name="w1t")
                        nc.sync.dma_start(
                            w1t, w1_d.ap()[ct * P:(ct + 1) * P,
                                           mg * 512:(mg + 1) * 512])
                        wts.append(w1t)
                    pg = [ps() for _ in range(4)]
                    for ct in range(CT):
                        for ml in range(4):
                            nc.tensor.matmul(
                                pg[ml], wts[ct][:, ml * P:(ml + 1) * P],
                                x1bf[:, ct, :],
                                start=(ct == 0), stop=False)
                    for ml in range(4):
                        mt = mg * 4 + ml
                        nc.tensor.matmul(
                            pg[ml], ns1g[0:1, ml * P:(ml + 1) * P], murow2,
                            start=False, stop=True)
                        zt = zp.tile([P, TOK], BF16, tag="zt", name="zt")
                        nc.vector.tensor_tensor(zt, pg[ml], rstd2_rep, ALU.mult)
                        nc.scalar.activation(gT[:, mt, :], zt, AF.Gelu_apprx_tanh)

                # matmul2 + residual -> outT
                for co in range(CT):
                    w2t = w2p.tile([P, MT, P], BF16, tag="w2t", name="w2t")
                    nc.scalar.dma_start(
                        w2t, w2r_d.ap()[co].rearrange("mo p c -> p mo c"))
                    po = ps()
                    for mt in range(MT):
                        nc.tensor.matmul(po, w2t[:, mt, :], gT[:, mt, :],
                                         start=(mt == 0), stop=(mt == MT - 1))
                    x1r = x1p.tile([P, TOK], F32, tag="x1o", name="x1o")
                    nc.scalar.dma_start(x1r, x1_spill[co * P:(co + 1) * P, :])
                    ot2 = outp.tile([P, TOK], F32, tag="ot2", name="ot2")
                    nc.vector.tensor_tensor(ot2, po, x1r, ALU.add)
                    nc.scalar.dma_start(out_d.ap()[co * P:(co + 1) * P, :], ot2)

    nc.compile()
    return nc


def _host_prep(x, w_qkv, w1, w2, ln_w):
    x = np.asarray(x, dtype=np.float32)
    w_qkv = np.asarray(w_qkv, dtype=np.float32)
    w1 = np.asarray(w1, dtype=np.float32)
    w2 = np.asarray(w2, dtype=np.float32)
    ln_w = np.asarray(ln_w, dtype=np.float32)

    xT = np.ascontiguousarray(x.transpose(0, 2, 1))            # [B, C, T]
    x_flat = x.reshape(B * T, C)

    Wq = (ln_w[:, None] * w_qkv[:, 0 * C:1 * C]) * SCALE
    Wk = ln_w[:, None] * w_qkv[:, 1 * C:2 * C]
    Wv = ln_w[:, None] * w_qkv[:, 2 * C:3 * C]
    nsq_full = -Wq.sum(0, dtype=np.float64).astype(np.float32)
    nsk_full = -Wk.sum(0, dtype=np.float64).astype(np.float32)
    nsv_full = -Wv.sum(0, dtype=np.float64).astype(np.float32)

    w1s = ln_w[:, None] * w1
    w1_bf = w1s.astype(ml_dtypes.bfloat16)
    ns1 = -w1s.sum(0, dtype=np.float64).astype(np.float32)
    # w2 reordered: [CT, MT, P(m), P(c)]
    w2r = np.ascontiguousarray(
        w2.reshape(MT, P, CT, P).transpose(2, 0, 1, 3)).astype(ml_dtypes.bfloat16)

    masks = np.zeros((4, P, 512), np.float32)
    for m in range(4):
        s_idx = np.arange(P)[:, None] + P * m
        t_idx = np.arange(512)[None, :]
        masks[m] = (t_idx >= s_idx).astype(np.float32)
    masks = masks.astype(ml_dtypes.bfloat16)

    in_maps = []
    for r in range(R):
        cs = slice(256 * r, 256 * (r + 1))
        b_own, tb_own = r // NT, r % NT
        in_maps.append({
            "xT": xT,
            "x_own": np.ascontiguousarray(x_flat[TOK * r: TOK * (r + 1)]),
            "xT_own": np.ascontiguousarray(
                xT[b_own][:, 512 * tb_own: 512 * (tb_own + 1)]),
            "wq": np.ascontiguousarray(Wq[:, cs]),
            "wk": np.ascontiguousarray(Wk[:, cs]),
            "wv": np.ascontiguousarray(Wv[:, cs]),
            "nsq": np.ascontiguousarray(nsq_full[cs]),
            "nsk": np.ascontiguousarray(nsk_full[cs]),
            "nsv": np.ascontiguousarray(nsv_full[cs]),
            "w1": w1_bf,
            "ns1": ns1,
            "w2r": w2r,
            "masks": masks,
        })
    return in_maps


def get_nc():
    if "nc" not in _CACHE:
        _CACHE["nc"] = _build()
    return _CACHE["nc"]


def run(in_maps, **kw):
    nc = get_nc()
    return run_bass_kernel_spmd(nc, in_maps, core_ids=list(range(R)), **kw)


def kernel(x, w_qkv, w1, w2, ln_w, **kw_unused):
    in_maps = _host_prep(x, w_qkv, w1, w2, ln_w)
    res = run(in_maps)
    out_flat = np.empty((B * T, C), np.float32)
    for r in range(R):
        out_flat[TOK * r: TOK * (r + 1)] = res.results[r]["outT"].T
    return out_flat.reshape(B, T, C)


# revision 14
# speedup vs baseline: 1.1177x; 1.0328x over previous
"""Trainium2 Bass kernel for nn_Block_10024453669245 (dense transformer block).

Strategy (8 NeuronCores):
  - warmup: dummy 32B AllGather prepays collective-communicator init.
  - Phase A: per-core LN1 stats on its 512 own tokens + tiny AllGather.
  - Phase B: QKV tensor-parallel over heads (2 heads/core). fp32r matmuls
    against host-transposed xT. LN1 is folded in: the rank-1 term
    (-colsum x murstd) is added via a K=1 fp32r matmul inside the same
    PSUM accumulation group; eviction is a single DVE mult by rstd.
    Produces qT,kT [d,t] and v [t,d] in bf16, resident in SBUF.
  - Phase C: causal attention head-major, no-max-sub softmax, S^T tiles,
    exp on ACT, causal masks on diagonal tiles, O^T and denominator
    accumulated on PE. Per-head AllToAll (2MB) fires as soon as that
    head's outputs are done, overlapping the other head's attention.
  - Phase E: MLP token-sharded (512 tokens/core) in bf16. ln2's weight is
    folded into w1 (host), the mean term via K=1 matmul fold, rstd2 at
    PSUM eviction. gelu = ACT Gelu_apprx_tanh. Residual from f32 spill.
    Output written transposed [C, 512] per core; host reassembles.
  DMAs are spread over sync/gpsimd/vector queues to avoid serializing.
"""
import sys, math

sys.path.insert(0, "/opt/trn_rl_repo")

import numpy as np
import ml_dtypes

import concourse.bass as bass
import concourse.tile as tile
from concourse import bacc, mybir
from concourse.bass_utils import run_bass_kernel_spmd

# ---------------- constants (hardcoded problem shape) ----------------
P = 128
B, T, C = 2, 2048, 2048
H, D = 16, 128
R = 8                 # cores
HL = H // R           # heads per core
TOK = B * T // R      # own tokens per core
CT = C // P           # 16 c-tiles
NT = T // 512         # 4 t-blocks per batch
M1 = 4 * C            # 8192
MT = M1 // P          # 64 m-tiles
MG = 16               # m-groups of 4 m-tiles (512 cols) for matmul1
EPS = 1e-5
SCALE = 1.0 / math.sqrt(D)

F32 = mybir.dt.float32
F32R = mybir.dt.float32r
BF16 = mybir.dt.bfloat16
AF = mybir.ActivationFunctionType
ALU = mybir.AluOpType

_CACHE = {}
DEBUG = False


def _pbc(t, n_free):
    """partition-broadcast AP over a 1-D dram tile view."""
    return bass.AP(tensor=t.tensor, offset=t.offset, ap=[[0, P], [1, n_free]])


def _row(ap1d):
    return ap1d.rearrange("(o t) -> o t", o=1)


def _build():
    nc = bacc.Bacc("TRN2", target_bir_lowering=False, debug=False, num_devices=R)

    # ---------------- I/O ----------------
    xT_d = nc.dram_tensor("xT", [B, C, T], F32, kind="ExternalInput")
    x_own_d = nc.dram_tensor("x_own", [TOK, C], F32, kind="ExternalInput")
    xT_own_d = nc.dram_tensor("xT_own", [C, TOK], F32, kind="ExternalInput")
    wq_d = nc.dram_tensor("wq", [C, HL * D], F32, kind="ExternalInput")
    wk_d = nc.dram_tensor("wk", [C, HL * D], F32, kind="ExternalInput")
    wv_d = nc.dram_tensor("wv", [C, HL * D], F32, kind="ExternalInput")
    nsq_d = nc.dram_tensor("nsq", [HL * D], F32, kind="ExternalInput")
    nsk_d = nc.dram_tensor("nsk", [HL * D], F32, kind="ExternalInput")
    nsv_d = nc.dram_tensor("nsv", [HL * D], F32, kind="ExternalInput")
    w1_d = nc.dram_tensor("w1", [C, M1], BF16, kind="ExternalInput")
    ns1_d = nc.dram_tensor("ns1", [M1], F32, kind="ExternalInput")
    w2r_d = nc.dram_tensor("w2r", [CT, MT, P, P], BF16, kind="ExternalInput")
    masks_d = nc.dram_tensor("masks", [4, P, 512], BF16, kind="ExternalInput")
    out_d = nc.dram_tensor("outT", [C, TOK], F32, kind="ExternalOutput")

    with tile.TileContext(nc) as tc:
        with tc.tile_pool(name="dram", bufs=1, space="DRAM") as dram, \
             tc.tile_pool(name="psum", bufs=8, space="PSUM") as psum, \
             tc.tile_pool(name="singles", bufs=1) as singles:

            # internal DRAM
            warm_in = dram.tile([8], F32)
            warm_out = dram.tile([R, 8], F32)
            stats_loc = dram.tile([2, TOK], F32)
            stats_g = dram.tile([R, 2, TOK], F32)
            a2a_in = [dram.tile([R, P, 512], F32, name=f"a2a_in{h}")
                      for h in range(HL)]
            a2a_out = [dram.tile([R, P, 512], F32, name=f"a2a_out{h}")
                       for h in range(HL)]
            x1_spill = dram.tile([C, TOK], F32)
            mlp_stat_b = dram.tile([2, TOK], F32)

            def ps():
                return psum.tile([P, 512], F32, tag="ps", name="ps")

            # warmup collective: pays communicator init while phase A runs
            nc.gpsimd.collective_compute(
                "AllGather", ALU.bypass, replica_groups=[list(range(R))],
                ins=[warm_in.opt()], outs=[warm_out.opt()])

            # small constants
            eps_t = singles.tile([P, 1], F32)
            nc.vector.memset(eps_t, EPS)
            ones_bf = singles.tile([P, 1], BF16)
            nc.vector.memset(ones_bf, 1.0)
            ones_f32 = singles.tile([P, 1], F32)
            nc.vector.memset(ones_f32, 1.0)

            # =========== Phase B+C pools (opened early: weight DMAs
            # go out on three parallel queues before phase A traffic) =======
            _wpool_cm = tc.tile_pool(name="wqkv", bufs=1)
            wpool = _wpool_cm.__enter__()
            wq_t = wpool.tile([P, CT, HL * D], F32R)
            nc.sync.dma_start(
                wq_t, wq_d.ap().rearrange("(ko p) n -> p ko n", p=P).bitcast(F32R))
            wk_t = wpool.tile([P, CT, HL * D], F32R)
            nc.scalar.dma_start(
                wk_t, wk_d.ap().rearrange("(ko p) n -> p ko n", p=P).bitcast(F32R))
            wv_t = wpool.tile([P, CT, HL * D], F32R)
            nc.gpsimd.dma_start(
                wv_t, wv_d.ap().rearrange("(ko p) n -> p ko n", p=P).bitcast(F32R))
            nsq_t = wpool.tile([1, HL * D], F32R)
            nc.gpsimd.dma_start(nsq_t, _row(nsq_d.ap()).bitcast(F32R))
            nsk_t = wpool.tile([1, HL * D], F32R)
            nc.gpsimd.dma_start(nsk_t, _row(nsk_d.ap()).bitcast(F32R))
            nsv_t = wpool.tile([1, HL * D], F32R)
            nc.gpsimd.dma_start(nsv_t, _row(nsv_d.ap()).bitcast(F32R))
            masks_t = wpool.tile([P, 4, 512], BF16)
            nc.scalar.dma_start(
                masks_t, masks_d.ap().rearrange("m p t -> p m t"))

            # =========== Phase A: LN1 stats on own tokens ===========
            with tc.tile_pool(name="stA", bufs=3) as stA:
                for i in range(TOK // P):
                    xo = stA.tile([P, C], F32, tag="xo", name="xo")
                    nc.gpsimd.dma_start(xo, x_own_d.ap()[i * P:(i + 1) * P, :])
                    xr = xo.rearrange("p (g s) -> p g s", s=512)
                    st = stA.tile([P, 4, 6], F32, tag="st", name="st")
                    for g in range(4):
                        nc.vector.bn_stats(out=st[:, g, :], in_=xr[:, g, :])
                    mv = stA.tile([P, 2], F32, tag="mv", name="mv")
                    nc.vector.bn_aggr(out=mv, in_=st)
                    rstd = stA.tile([P, 1], F32, tag="rstd", name="rstd")
                    nc.scalar.activation(rstd, mv[:, 1:2], AF.Sqrt, bias=eps_t)
                    nc.vector.reciprocal_approx_fast(out=rstd, in_=rstd)
                    murstd = stA.tile([P, 1], F32, tag="murstd", name="murstd")
                    nc.vector.tensor_tensor(murstd, mv[:, 0:1], rstd, ALU.mult)
                    nc.gpsimd.dma_start(
                        stats_loc[0, i * P:(i + 1) * P].rearrange("(p o) -> p o", o=1),
                        rstd)
                    nc.gpsimd.dma_start(
                        stats_loc[1, i * P:(i + 1) * P].rearrange("(p o) -> p o", o=1),
                        murstd)
            nc.gpsimd.collective_compute(
                "AllGather", ALU.bypass,
                replica_groups=[list(range(R))],
                ins=[stats_loc.opt()], outs=[stats_g.opt()])

            # =========== Phase B+C pools ===========
            with tc.tile_pool(name="qkvres", bufs=1) as qkvres, \
                 tc.tile_pool(name="xtp", bufs=4) as xtp, \
                 tc.tile_pool(name="reps", bufs=4) as reps, \
                 tc.tile_pool(name="tmps", bufs=4) as tmps, \
                 tc.tile_pool(name="attn", bufs=3) as attnp:

                # persistent qkv (bf16)
                qT = [[qkvres.tile([P, T], BF16, name=f"qT{h}{b}")
                       for b in range(B)] for h in range(HL)]
                kT = [[qkvres.tile([P, T], BF16, name=f"kT{h}{b}")
                       for b in range(B)] for h in range(HL)]
                vsb = [qkvres.tile([P, T // P, HL * D], BF16, name=f"v{b}")
                       for b in range(B)]

                # =========== Phase B: QKV ===========
                for b in range(B):
                    for tb in range(NT):
                        j = NT * b + tb
                        t0 = 512 * tb
                        murow = reps.tile([1, 512], F32R, tag="murow", name="murow")
                        nc.gpsimd.dma_start(
                            murow, _row(stats_g[j, 1, :]).bitcast(F32R))
                        rstd_rep = reps.tile([P, 512], F32, tag="rrep", name="rstd_rep")
                        nc.gpsimd.dma_start(rstd_rep, _pbc(stats_g[j, 0, :], 512))

                        pq = [ps() for _ in range(HL)]
                        pk = [ps() for _ in range(HL)]
                        # one bank per 128-token v subtile (start=True clears
                        # the whole bank, chains must not share one)
                        pv = [ps() for _ in range(4)]
                        for ko in range(CT):
                            xt = xtp.tile([P, 512], F32R, tag="xt", name="xt")
                            nc.sync.dma_start(
                                xt,
                                xT_d.ap()[b, ko * P:(ko + 1) * P,
                                          t0:t0 + 512].bitcast(F32R))
                            st_flag = ko == 0
                            for hl in range(HL):
                                nc.tensor.matmul(
                                    pq[hl], wq_t[:, ko, hl * D:(hl + 1) * D], xt,
                                    start=st_flag, stop=False)
                                nc.tensor.matmul(
                                    pk[hl], wk_t[:, ko, hl * D:(hl + 1) * D], xt,
                                    start=st_flag, stop=False)
                            for ss in range(4):
                                nc.tensor.matmul(
                                    pv[ss][:, 0:256],
                                    xt[:, ss * P:(ss + 1) * P], wv_t[:, ko, :],
                                    start=st_flag, stop=False)
                        # rank-1 LN fold: += (-colsum) x murstd  (K=1 matmul)
                        for hl in range(HL):
                            nc.tensor.matmul(
                                pq[hl], nsq_t[0:1, hl * D:(hl + 1) * D], murow,
                                start=False, stop=True)
                            nc.tensor.matmul(
                                pk[hl], nsk_t[0:1, hl * D:(hl + 1) * D], murow,
                                start=False, stop=True)
                        for ss in range(4):
                            nc.tensor.matmul(
                                pv[ss][:, 0:256],
                                murow[0:1, ss * P:(ss + 1) * P], nsv_t,
                                start=False, stop=True)
                        # evictions: single mult by rstd
                        for hl in range(HL):
                            nc.vector.tensor_tensor(
                                qT[hl][b][:, t0:t0 + 512], pq[hl], rstd_rep, ALU.mult)
                            nc.vector.tensor_tensor(
                                kT[hl][b][:, t0:t0 + 512], pk[hl], rstd_rep, ALU.mult)
                        for ss in range(4):
                            si = tb * 4 + ss
                            rstd_c = tmps.tile([P, 1], F32, tag="rc", name="rc")
                            nc.gpsimd.dma_start(
                                rstd_c,
                                stats_g[j, 0, ss * P:(ss + 1) * P]
                                .rearrange("(p o) -> p o", o=1))
                            nc.vector.tensor_scalar(
                                vsb[b][:, si, :], pv[ss][:, 0:256], rstd_c, None,
                                ALU.mult)

                # =========== Phase C: attention (head-major) ===========
                for hl in range(HL):
                    for b in range(B):
                        for tb in range(NT):
                            t0 = 512 * tb
                            n_s = 4 * (tb + 1)
                            pot = ps()
                            pden = ps()
                            for si in range(n_s):
                                pS = ps()
                                nc.tensor.matmul(
                                    pS, kT[hl][b][:, si * P:(si + 1) * P],
                                    qT[hl][b][:, t0:t0 + 512],
                                    start=True, stop=True)
                                pt = attnp.tile([P, 512], BF16, tag="pt", name="pt")
                                nc.scalar.activation(pt, pS, AF.Exp)
                                m = si - (n_s - 4)
                                if m >= 0:
                                    nc.vector.tensor_tensor(
                                        pt, pt, masks_t[:, m, :], ALU.mult)
                                nc.tensor.matmul(
                                    pot, vsb[b][:, si, hl * D:(hl + 1) * D], pt,
                                    start=(si == 0), stop=(si == n_s - 1))
                                nc.tensor.matmul(
                                    pden[0:1, :], ones_bf, pt,
                                    start=(si == 0), stop=(si == n_s - 1))
                            # normalize: OT / den
                            den_r = attnp.tile([1, 512], F32, tag="dr", name="den_r")
                            nc.vector.reciprocal_approx_fast(out=den_r, in_=pden[0:1, :])
                            den_d = dram.tile([512], F32, tag="den_d", bufs=4,
                                              name="den_d")
                            nc.gpsimd.dma_start(_row(den_d), den_r)
                            den_rep = reps.tile([P, 512], F32, tag="denrep",
                                                name="den_rep")
                            nc.gpsimd.dma_start(den_rep, _pbc(den_d, 512))
                            ot = attnp.tile([P, 512], F32, tag="ot", name="ot")
                            nc.vector.tensor_tensor(ot, pot, den_rep, ALU.mult)
                            nc.gpsimd.dma_start(a2a_in[hl][NT * b + tb, :, :], ot)
                    # per-head AllToAll fires as soon as head hl is done
                    nc.gpsimd.collective_compute(
                        "AllToAll", ALU.bypass,
                        replica_groups=[list(range(R))],
                        ins=[a2a_in[hl].opt()], outs=[a2a_out[hl].opt()])

            _wpool_cm.__exit__(None, None, None)

            # =========== Phase E: MLP (token-sharded, bf16) ===========
            with tc.tile_pool(name="mlp_x1", bufs=3) as x1p, \
                 tc.tile_pool(name="mlp_sq", bufs=2) as sqp, \
                 tc.tile_pool(name="mlp_x1bf", bufs=1) as x1bfp, \
                 tc.tile_pool(name="mlp_g", bufs=1) as gp, \
                 tc.tile_pool(name="mlp_w1", bufs=22) as w1p, \
                 tc.tile_pool(name="mlp_w2", bufs=2) as w2p, \
                 tc.tile_pool(name="mlp_z", bufs=4) as zp, \
                 tc.tile_pool(name="mlp_out", bufs=2) as outp:

                x1bf = x1bfp.tile([P, CT, TOK], BF16)
                # pass 1: build x1 tiles (evens first: only need a2a head 0),
                # stats matmuls, bf16 copy, f32 spill
                pmu = ps()
                psq = ps()
                order = [2 * i for i in range(CT // 2)] + \
                        [2 * i + 1 for i in range(CT // 2)]
                for idx, ct in enumerate(order):
                    xo = x1p.tile([P, TOK], F32, tag="xo2", name="xo2")
                    nc.gpsimd.dma_start(xo, xT_own_d.ap()[ct * P:(ct + 1) * P, :])
                    at = x1p.tile([P, TOK], F32, tag="at", name="at")
                    nc.gpsimd.dma_start(at, a2a_out[ct % 2][ct // 2])
                    x1 = x1p.tile([P, TOK], F32, tag="x1", name="x1")
                    nc.vector.tensor_tensor(x1, xo, at, ALU.add)
                    nc.gpsimd.dma_start(x1_spill[ct * P:(ct + 1) * P, :], x1)
                    nc.vector.tensor_copy(x1bf[:, ct, :], x1)
                    sq2 = sqp.tile([P, TOK], BF16, tag="sq2", name="sq2")
                    nc.vector.tensor_tensor(sq2, x1bf[:, ct, :], x1bf[:, ct, :],
                                            ALU.mult)
                    nc.tensor.matmul(pmu[0:1, :], ones_bf, x1bf[:, ct, :],
                                     start=(idx == 0), stop=(idx == CT - 1))
                    nc.tensor.matmul(psq[0:1, :], ones_bf, sq2,
                                     start=(idx == 0), stop=(idx == CT - 1))
                # finalize stats: mu = pmu/C ; var = psq/C - mu^2
                mu2 = singles.tile([1, TOK], F32)
                nc.vector.tensor_scalar(mu2, pmu[0:1, :], 1.0 / C, None, ALU.mult)
                var2 = singles.tile([1, TOK], F32)
                nc.vector.tensor_scalar(var2, psq[0:1, :], 1.0 / C, None, ALU.mult)
                musq = singles.tile([1, TOK], F32)
                nc.vector.tensor_tensor(musq, mu2, mu2, ALU.mult)
                nc.vector.tensor_tensor(var2, var2, musq, ALU.subtract)
                rstd2 = singles.tile([1, TOK], F32)
                nc.scalar.activation(rstd2, var2, AF.Sqrt, bias=eps_t[0:1])
                nc.vector.reciprocal_approx_fast(out=rstd2, in_=rstd2)
                nc.gpsimd.dma_start(_row(mlp_stat_b[0, :]), mu2)
                nc.gpsimd.dma_start(_row(mlp_stat_b[1, :]), rstd2)
                murow2 = singles.tile([1, TOK], F32R)
                nc.gpsimd.dma_start(murow2, _row(mlp_stat_b[0, :]).bitcast(F32R))
                rstd2_rep = singles.tile([P, TOK], F32)
                nc.gpsimd.dma_start(rstd2_rep, _pbc(mlp_stat_b[1, :], TOK))

                # matmul1 (+ mean fold) -> *rstd2 -> gelu -> gT
                gT = gp.tile([P, MT, TOK], BF16)
                for mg in range(MG):
                    ns1g = zp.tile([1, 512], F32R, tag="ns1g", name="ns1g")
                    nc.gpsimd.dma_start(
                        ns1g, _row(ns1_d.ap()[mg * 512:(mg + 1) * 512]).bitcast(F32R))
                    wts = []
                    for ct in range(CT):
                        w1t = w1p.tile([P, 512], BF16, tag="w1t", name="w1t")
                        nc.sync.dma_start(
                            w1t, w1_d.ap()[ct * P:(ct + 1) * P,
                                           mg * 512:(mg + 1) * 512])
                        wts.append(w1t)
                    pg = [ps() for _ in range(4)]
                    for ci, ct in enumerate(order):
                        for ml in range(4):
                            nc.tensor.matmul(
                                pg[ml], wts[ct][:, ml * P:(ml + 1) * P],
                                x1bf[:, ct, :],
                                start=(ci == 0), stop=False)
                    for ml in range(4):
                        mt = mg * 4 + ml
                        nc.tensor.matmul(
                            pg[ml], ns1g[0:1, ml * P:(ml + 1) * P], murow2,
                            start=False, stop=True)
                        zt = zp.tile([P, TOK], BF16, tag="zt", name="zt")
                        nc.vector.tensor_tensor(zt, pg[ml], rstd2_rep, ALU.mult)
                        nc.scalar.activation(gT[:, mt, :], zt, AF.Gelu_apprx_tanh)

                # matmul2 + residual -> outT
                for co in range(CT):
                    w2t = w2p.tile([P, MT, P], BF16, tag="w2t", name="w2t")
                    nc.scalar.dma_start(
                        w2t, w2r_d.ap()[co].rearrange("mo p c -> p mo c"))
                    po = ps()
                    for mt in range(MT):
                        nc.tensor.matmul(po, w2t[:, mt, :], gT[:, mt, :],
                                         start=(mt == 0), stop=(mt == MT - 1))
                    x1r = x1p.tile([P, TOK], F32, tag="x1o", name="x1o")
                    nc.scalar.dma_start(x1r, x1_spill[co * P:(co + 1) * P, :])
                    ot2 = outp.tile([P, TOK], F32, tag="ot2", name="ot2")
                    nc.vector.tensor_tensor(ot2, po, x1r, ALU.add)
                    nc.scalar.dma_start(out_d.ap()[co * P:(co + 1) * P, :], ot2)

    nc.compile()
    return nc


def _host_prep(x, w_qkv, w1, w2, ln_w):
    x = np.asarray(x, dtype=np.float32)
    w_qkv = np.asarray(w_qkv, dtype=np.float32)
    w1 = np.asarray(w1, dtype=np.float32)
    w2 = np.asarray(w2, dtype=np.float32)
    ln_w = np.asarray(ln_w, dtype=np.float32)

    xT = np.ascontiguousarray(x.transpose(0, 2, 1))            # [B, C, T]
    x_flat = x.reshape(B * T, C)

    Wq = (ln_w[:, None] * w_qkv[:, 0 * C:1 * C]) * SCALE
    Wk = ln_w[:, None] * w_qkv[:, 1 * C:2 * C]
    Wv = ln_w[:, None] * w_qkv[:, 2 * C:3 * C]
    nsq_full = -Wq.sum(0, dtype=np.float64).astype(np.float32)
    nsk_full = -Wk.sum(0, dtype=np.float64).astype(np.float32)
    nsv_full = -Wv.sum(0, dtype=np.float64).astype(np.float32)

    w1s = ln_w[:, None] * w1
    w1_bf = w1s.astype(ml_dtypes.bfloat16)
    ns1 = -w1s.sum(0, dtype=np.float64).astype(np.float32)
    # w2 reordered: [CT, MT, P(m), P(c)]
    w2r = np.ascontiguousarray(
        w2.reshape(MT, P, CT, P).transpose(2, 0, 1, 3)).astype(ml_dtypes.bfloat16)

    masks = np.zeros((4, P, 512), np.float32)
    for m in range(4):
        s_idx = np.arange(P)[:, None] + P * m
        t_idx = np.arange(512)[None, :]
        masks[m] = (t_idx >= s_idx).astype(np.float32)
    masks = masks.astype(ml_dtypes.bfloat16)

    in_maps = []
    for r in range(R):
        cs = slice(256 * r, 256 * (r + 1))
        b_own, tb_own = r // NT, r % NT
        in_maps.append({
            "xT": xT,
            "x_own": np.ascontiguousarray(x_flat[TOK * r: TOK * (r + 1)]),
            "xT_own": np.ascontiguousarray(
                xT[b_own][:, 512 * tb_own: 512 * (tb_own + 1)]),
            "wq": np.ascontiguousarray(Wq[:, cs]),
            "wk": np.ascontiguousarray(Wk[:, cs]),
            "wv": np.ascontiguousarray(Wv[:, cs]),
            "nsq": np.ascontiguousarray(nsq_full[cs]),
            "nsk": np.ascontiguousarray(nsk_full[cs]),
            "nsv": np.ascontiguousarray(nsv_full[cs]),
            "w1": w1_bf,
            "ns1": ns1,
            "w2r": w2r,
            "masks": masks,
        })
    return in_maps


def get_nc():
    if "nc" not in _CACHE:
        _CACHE["nc"] = _build()
    return _CACHE["nc"]


def run(in_maps, **kw):
    nc = get_nc()
    return run_bass_kernel_spmd(nc, in_maps, core_ids=list(range(R)), **kw)


def kernel(x, w_qkv, w1, w2, ln_w, **kw_unused):
    in_maps = _host_prep(x, w_qkv, w1, w2, ln_w)
    res = run(in_maps)
    out_flat = np.empty((B * T, C), np.float32)
    for r in range(R):
        out_flat[TOK * r: TOK * (r + 1)] = res.results[r]["outT"].T
    return out_flat.reshape(B, T, C)


# revision 15
# speedup vs baseline: 1.1572x; 1.0353x over previous
"""Trainium2 Bass kernel for nn_Block_10024453669245 (dense transformer block).

Strategy (8 NeuronCores):
  - warmup: dummy 32B AllGather prepays collective-communicator init.
  - Phase A: per-core LN1 stats on its 512 own tokens + tiny AllGather.
  - Phase B: QKV tensor-parallel over heads (2 heads/core). fp32r matmuls
    against host-transposed xT. LN1 is folded in: the rank-1 term
    (-colsum x murstd) is added via a K=1 fp32r matmul inside the same
    PSUM accumulation group; eviction is a single DVE mult by rstd.
    Produces qT,kT [d,t] and v [t,d] in bf16, resident in SBUF.
  - Phase C: causal attention head-major, no-max-sub softmax, S^T tiles,
    exp on ACT, causal masks on diagonal tiles, O^T and denominator
    accumulated on PE. Per-head AllToAll (2MB) fires as soon as that
    head's outputs are done, overlapping the other head's attention.
  - Phase E: MLP token-sharded (512 tokens/core) in bf16. ln2's weight is
    folded into w1 (host), the mean term via K=1 matmul fold, rstd2 at
    PSUM eviction. gelu = ACT Gelu_apprx_tanh. Residual from f32 spill.
    Output written transposed [C, 512] per core; host reassembles.
  DMAs are spread over sync/gpsimd/vector queues to avoid serializing.
"""
import sys, math

sys.path.insert(0, "/opt/trn_rl_repo")

import numpy as np
import ml_dtypes

import concourse.bass as bass
import concourse.tile as tile
from concourse import bacc, mybir
from concourse.bass_utils import run_bass_kernel_spmd

# ---------------- constants (hardcoded problem shape) ----------------
P = 128
B, T, C = 2, 2048, 2048
H, D = 16, 128
R = 8                 # cores
HL = H // R           # heads per core
TOK = B * T // R      # own tokens per core
CT = C // P           # 16 c-tiles
NT = T // 512         # 4 t-blocks per batch
M1 = 4 * C            # 8192
MT = M1 // P          # 64 m-tiles
MG = 16               # m-groups of 4 m-tiles (512 cols) for matmul1
EPS = 1e-5
SCALE = 1.0 / math.sqrt(D)

F32 = mybir.dt.float32
F32R = mybir.dt.float32r
BF16 = mybir.dt.bfloat16
AF = mybir.ActivationFunctionType
ALU = mybir.AluOpType

_CACHE = {}
DEBUG = False


def _pbc(t, n_free):
    """partition-broadcast AP over a 1-D dram tile view."""
    return bass.AP(tensor=t.tensor, offset=t.offset, ap=[[0, P], [1, n_free]])


def _row(ap1d):
    return ap1d.rearrange("(o t) -> o t", o=1)


def _build():
    nc = bacc.Bacc("TRN2", target_bir_lowering=False, debug=False, num_devices=R)

    # ---------------- I/O ----------------
    xT_d = nc.dram_tensor("xT", [B, C, T], BF16, kind="ExternalInput")
    x_own_d = nc.dram_tensor("x_own", [TOK, C], F32, kind="ExternalInput")
    xT_own_d = nc.dram_tensor("xT_own", [C, TOK], F32, kind="ExternalInput")
    wq_d = nc.dram_tensor("wq", [C, HL * D], BF16, kind="ExternalInput")
    wk_d = nc.dram_tensor("wk", [C, HL * D], BF16, kind="ExternalInput")
    wv_d = nc.dram_tensor("wv", [C, HL * D], BF16, kind="ExternalInput")
    nsq_d = nc.dram_tensor("nsq", [HL * D], BF16, kind="ExternalInput")
    nsk_d = nc.dram_tensor("nsk", [HL * D], BF16, kind="ExternalInput")
    nsv_d = nc.dram_tensor("nsv", [HL * D], BF16, kind="ExternalInput")
    w1_d = nc.dram_tensor("w1", [C, M1], BF16, kind="ExternalInput")
    ns1_d = nc.dram_tensor("ns1", [M1], BF16, kind="ExternalInput")
    w2r_d = nc.dram_tensor("w2r", [CT, MT, P, P], BF16, kind="ExternalInput")
    masks_d = nc.dram_tensor("masks", [4, P, 512], BF16, kind="ExternalInput")
    out_d = nc.dram_tensor("outT", [C, TOK], F32, kind="ExternalOutput")

    with tile.TileContext(nc) as tc:
        with tc.tile_pool(name="dram", bufs=1, space="DRAM") as dram, \
             tc.tile_pool(name="psum", bufs=8, space="PSUM") as psum, \
             tc.tile_pool(name="singles", bufs=1) as singles:

            # internal DRAM
            warm_in = dram.tile([8], F32)
            warm_out = dram.tile([R, 8], F32)
            stats_loc = dram.tile([2, TOK], F32)
            stats_g = dram.tile([R, 2, TOK], F32)
            a2a_in = [dram.tile([R, P, 512], F32, name=f"a2a_in{h}")
                      for h in range(HL)]
            a2a_out = [dram.tile([R, P, 512], F32, name=f"a2a_out{h}")
                       for h in range(HL)]
            x1_spill = dram.tile([C, TOK], F32)
            mlp_stat_b = dram.tile([2, TOK], F32)

            def ps():
                return psum.tile([P, 512], F32, tag="ps", name="ps")

            # warmup collective: pays communicator init while phase A runs
            nc.gpsimd.collective_compute(
                "AllGather", ALU.bypass, replica_groups=[list(range(R))],
                ins=[warm_in.opt()], outs=[warm_out.opt()])

            # small constants
            eps_t = singles.tile([P, 1], F32)
            nc.vector.memset(eps_t, EPS)
            ones_bf = singles.tile([P, 1], BF16)
            nc.vector.memset(ones_bf, 1.0)
            ones_f32 = singles.tile([P, 1], F32)
            nc.vector.memset(ones_f32, 1.0)

            # =========== Phase B+C pools (opened early: weight DMAs
            # go out on three parallel queues before phase A traffic) =======
            _wpool_cm = tc.tile_pool(name="wqkv", bufs=1)
            wpool = _wpool_cm.__enter__()
            wq_t = wpool.tile([P, CT, HL * D], BF16)
            nc.sync.dma_start(
                wq_t, wq_d.ap().rearrange("(ko p) n -> p ko n", p=P))
            wk_t = wpool.tile([P, CT, HL * D], BF16)
            nc.scalar.dma_start(
                wk_t, wk_d.ap().rearrange("(ko p) n -> p ko n", p=P))
            wv_t = wpool.tile([P, CT, HL * D], BF16)
            nc.sync.dma_start(
                wv_t, wv_d.ap().rearrange("(ko p) n -> p ko n", p=P))
            nsq_t = wpool.tile([1, HL * D], BF16)
            nc.gpsimd.dma_start(nsq_t, _row(nsq_d.ap()))
            nsk_t = wpool.tile([1, HL * D], BF16)
            nc.gpsimd.dma_start(nsk_t, _row(nsk_d.ap()))
            nsv_t = wpool.tile([1, HL * D], BF16)
            nc.gpsimd.dma_start(nsv_t, _row(nsv_d.ap()))
            masks_t = wpool.tile([P, 4, 512], BF16)
            nc.scalar.dma_start(
                masks_t, masks_d.ap().rearrange("m p t -> p m t"))

            # =========== Phase A: LN1 stats on own tokens ===========
            with tc.tile_pool(name="stA", bufs=3) as stA:
                for i in range(TOK // P):
                    xo = stA.tile([P, C], F32, tag="xo", name="xo")
                    nc.gpsimd.dma_start(xo, x_own_d.ap()[i * P:(i + 1) * P, :])
                    xr = xo.rearrange("p (g s) -> p g s", s=512)
                    st = stA.tile([P, 4, 6], F32, tag="st", name="st")
                    for g in range(4):
                        nc.vector.bn_stats(out=st[:, g, :], in_=xr[:, g, :])
                    mv = stA.tile([P, 2], F32, tag="mv", name="mv")
                    nc.vector.bn_aggr(out=mv, in_=st)
                    rstd = stA.tile([P, 1], F32, tag="rstd", name="rstd")
                    nc.scalar.activation(rstd, mv[:, 1:2], AF.Sqrt, bias=eps_t)
                    nc.vector.reciprocal_approx_fast(out=rstd, in_=rstd)
                    murstd = stA.tile([P, 1], F32, tag="murstd", name="murstd")
                    nc.vector.tensor_tensor(murstd, mv[:, 0:1], rstd, ALU.mult)
                    nc.gpsimd.dma_start(
                        stats_loc[0, i * P:(i + 1) * P].rearrange("(p o) -> p o", o=1),
                        rstd)
                    nc.gpsimd.dma_start(
                        stats_loc[1, i * P:(i + 1) * P].rearrange("(p o) -> p o", o=1),
                        murstd)
            nc.gpsimd.collective_compute(
                "AllGather", ALU.bypass,
                replica_groups=[list(range(R))],
                ins=[stats_loc.opt()], outs=[stats_g.opt()])

            # =========== Phase B+C pools ===========
            with tc.tile_pool(name="qkvres", bufs=1) as qkvres, \
                 tc.tile_pool(name="xtp", bufs=8) as xtp, \
                 tc.tile_pool(name="reps", bufs=4) as reps, \
                 tc.tile_pool(name="tmps", bufs=4) as tmps, \
                 tc.tile_pool(name="attn", bufs=3) as attnp:

                # persistent qkv (bf16)
                qT = [[qkvres.tile([P, T], BF16, name=f"qT{h}{b}")
                       for b in range(B)] for h in range(HL)]
                kT = [[qkvres.tile([P, T], BF16, name=f"kT{h}{b}")
                       for b in range(B)] for h in range(HL)]
                vsb = [qkvres.tile([P, T // P, HL * D], BF16, name=f"v{b}")
                       for b in range(B)]

                # =========== Phase B: QKV ===========
                for b in range(B):
                    for tb in range(NT):
                        j = NT * b + tb
                        t0 = 512 * tb
                        murow_f = reps.tile([1, 512], F32, tag="murowf", name="murow_f")
                        nc.gpsimd.dma_start(murow_f, _row(stats_g[j, 1, :]))
                        murow = reps.tile([1, 512], BF16, tag="murow", name="murow")
                        nc.vector.tensor_copy(murow, murow_f)
                        rstd_rep = reps.tile([P, 512], F32, tag="rrep", name="rstd_rep")
                        nc.gpsimd.dma_start(rstd_rep, _pbc(stats_g[j, 0, :], 512))

                        pq = [ps() for _ in range(HL)]
                        pk = [ps() for _ in range(HL)]
                        # one bank per 128-token v subtile (start=True clears
                        # the whole bank, chains must not share one)
                        pv = [ps() for _ in range(4)]
                        for ko in range(CT):
                            xt = xtp.tile([P, 512], BF16, tag="xt", name="xt")
                            nc.sync.dma_start(
                                xt,
                                xT_d.ap()[b, ko * P:(ko + 1) * P, t0:t0 + 512])
                            st_flag = ko == 0
                            for hl in range(HL):
                                nc.tensor.matmul(
                                    pq[hl], wq_t[:, ko, hl * D:(hl + 1) * D], xt,
                                    start=st_flag, stop=False)
                                nc.tensor.matmul(
                                    pk[hl], wk_t[:, ko, hl * D:(hl + 1) * D], xt,
                                    start=st_flag, stop=False)
                            for ss in range(4):
                                nc.tensor.matmul(
                                    pv[ss][:, 0:256],
                                    xt[:, ss * P:(ss + 1) * P], wv_t[:, ko, :],
                                    start=st_flag, stop=False)
                        # rank-1 LN fold: += (-colsum) x murstd  (K=1 matmul)
                        for hl in range(HL):
                            nc.tensor.matmul(
                                pq[hl], nsq_t[0:1, hl * D:(hl + 1) * D], murow,
                                start=False, stop=True)
                            nc.tensor.matmul(
                                pk[hl], nsk_t[0:1, hl * D:(hl + 1) * D], murow,
                                start=False, stop=True)
                        for ss in range(4):
                            nc.tensor.matmul(
                                pv[ss][:, 0:256],
                                murow[0:1, ss * P:(ss + 1) * P], nsv_t,
                                start=False, stop=True)
                        # evictions: single mult by rstd
                        for hl in range(HL):
                            nc.vector.tensor_tensor(
                                qT[hl][b][:, t0:t0 + 512], pq[hl], rstd_rep, ALU.mult)
                            nc.vector.tensor_tensor(
                                kT[hl][b][:, t0:t0 + 512], pk[hl], rstd_rep, ALU.mult)
                        for ss in range(4):
                            si = tb * 4 + ss
                            rstd_c = tmps.tile([P, 1], F32, tag="rc", name="rc")
                            nc.gpsimd.dma_start(
                                rstd_c,
                                stats_g[j, 0, ss * P:(ss + 1) * P]
                                .rearrange("(p o) -> p o", o=1))
                            nc.vector.tensor_scalar(
                                vsb[b][:, si, :], pv[ss][:, 0:256], rstd_c, None,
                                ALU.mult)

                # =========== Phase C: attention (head-major) ===========
                for hl in range(HL):
                    for b in range(B):
                        for tb in range(NT):
                            t0 = 512 * tb
                            n_s = 4 * (tb + 1)
                            pot = ps()
                            pden = ps()
                            for si in range(n_s):
                                pS = ps()
                                nc.tensor.matmul(
                                    pS, kT[hl][b][:, si * P:(si + 1) * P],
                                    qT[hl][b][:, t0:t0 + 512],
                                    start=True, stop=True)
                                pt = attnp.tile([P, 512], BF16, tag="pt", name="pt")
                                nc.scalar.activation(pt, pS, AF.Exp)
                                m = si - (n_s - 4)
                                if m >= 0:
                                    nc.vector.tensor_tensor(
                                        pt, pt, masks_t[:, m, :], ALU.mult)
                                nc.tensor.matmul(
                                    pot, vsb[b][:, si, hl * D:(hl + 1) * D], pt,
                                    start=(si == 0), stop=(si == n_s - 1))
                                nc.tensor.matmul(
                                    pden[0:1, :], ones_bf, pt,
                                    start=(si == 0), stop=(si == n_s - 1))
                            # normalize: OT / den
                            den_r = attnp.tile([1, 512], F32, tag="dr", name="den_r")
                            nc.vector.reciprocal_approx_fast(out=den_r, in_=pden[0:1, :])
                            den_d = dram.tile([512], F32, tag="den_d", bufs=4,
                                              name="den_d")
                            nc.gpsimd.dma_start(_row(den_d), den_r)
                            den_rep = reps.tile([P, 512], F32, tag="denrep",
                                                name="den_rep")
                            nc.gpsimd.dma_start(den_rep, _pbc(den_d, 512))
                            ot = attnp.tile([P, 512], F32, tag="ot", name="ot")
                            nc.vector.tensor_tensor(ot, pot, den_rep, ALU.mult)
                            nc.gpsimd.dma_start(a2a_in[hl][NT * b + tb, :, :], ot)
                    # per-head AllToAll fires as soon as head hl is done
                    nc.gpsimd.collective_compute(
                        "AllToAll", ALU.bypass,
                        replica_groups=[list(range(R))],
                        ins=[a2a_in[hl].opt()], outs=[a2a_out[hl].opt()])

            _wpool_cm.__exit__(None, None, None)

            # =========== Phase E: MLP (token-sharded, bf16) ===========
            with tc.tile_pool(name="mlp_x1", bufs=3) as x1p, \
                 tc.tile_pool(name="mlp_sq", bufs=2) as sqp, \
                 tc.tile_pool(name="mlp_x1bf", bufs=1) as x1bfp, \
                 tc.tile_pool(name="mlp_g", bufs=1) as gp, \
                 tc.tile_pool(name="mlp_w1", bufs=22) as w1p, \
                 tc.tile_pool(name="mlp_w2", bufs=2) as w2p, \
                 tc.tile_pool(name="mlp_z", bufs=4) as zp, \
                 tc.tile_pool(name="mlp_out", bufs=2) as outp:

                x1bf = [x1bfp.tile([P, TOK], BF16, name=f"x1bf{i}")
                        for i in range(CT)]
                # pass 1: build x1 tiles (evens first: only need a2a head 0),
                # stats matmuls, bf16 copy, f32 spill
                pmu = ps()
                psq = ps()
                order = [2 * i for i in range(CT // 2)] + \
                        [2 * i + 1 for i in range(CT // 2)]
                for idx, ct in enumerate(order):
                    xo = x1p.tile([P, TOK], F32, tag="xo2", name="xo2")
                    nc.gpsimd.dma_start(xo, xT_own_d.ap()[ct * P:(ct + 1) * P, :])
                    at = x1p.tile([P, TOK], F32, tag="at", name="at")
                    nc.gpsimd.dma_start(at, a2a_out[ct % 2][ct // 2])
                    x1 = x1p.tile([P, TOK], F32, tag="x1", name="x1")
                    nc.vector.tensor_tensor(x1, xo, at, ALU.add)
                    nc.gpsimd.dma_start(x1_spill[ct * P:(ct + 1) * P, :], x1)
                    nc.vector.tensor_copy(x1bf[ct], x1)
                    sq2 = sqp.tile([P, TOK], BF16, tag="sq2", name="sq2")
                    nc.vector.tensor_tensor(sq2, x1bf[ct], x1bf[ct], ALU.mult)
                    nc.tensor.matmul(pmu[0:1, :], ones_bf, x1bf[ct],
                                     start=(idx == 0), stop=(idx == CT - 1))
                    nc.tensor.matmul(psq[0:1, :], ones_bf, sq2,
                                     start=(idx == 0), stop=(idx == CT - 1))
                # finalize stats: mu = pmu/C ; var = psq/C - mu^2
                mu2 = singles.tile([1, TOK], F32)
                nc.vector.tensor_scalar(mu2, pmu[0:1, :], 1.0 / C, None, ALU.mult)
                var2 = singles.tile([1, TOK], F32)
                nc.vector.tensor_scalar(var2, psq[0:1, :], 1.0 / C, None, ALU.mult)
                musq = singles.tile([1, TOK], F32)
                nc.vector.tensor_tensor(musq, mu2, mu2, ALU.mult)
                nc.vector.tensor_tensor(var2, var2, musq, ALU.subtract)
                rstd2 = singles.tile([1, TOK], F32)
                nc.scalar.activation(rstd2, var2, AF.Sqrt, bias=eps_t[0:1])
                nc.vector.reciprocal_approx_fast(out=rstd2, in_=rstd2)
                nc.gpsimd.dma_start(_row(mlp_stat_b[0, :]), mu2)
                nc.gpsimd.dma_start(_row(mlp_stat_b[1, :]), rstd2)
                murow2 = singles.tile([1, TOK], BF16)
                nc.vector.tensor_copy(murow2, mu2)
                rstd2_rep = singles.tile([P, TOK], F32)
                nc.gpsimd.dma_start(rstd2_rep, _pbc(mlp_stat_b[1, :], TOK))

                # matmul1 (+ mean fold) -> *rstd2 -> gelu -> gT
                gT = gp.tile([P, MT, TOK], BF16)
                for mg in range(MG):
                    ns1g = zp.tile([1, 512], BF16, tag="ns1g", name="ns1g")
                    nc.gpsimd.dma_start(
                        ns1g, _row(ns1_d.ap()[mg * 512:(mg + 1) * 512]))
                    wts = []
                    for ct in range(CT):
                        w1t = w1p.tile([P, 512], BF16, tag="w1t", name="w1t")
                        nc.sync.dma_start(
                            w1t, w1_d.ap()[ct * P:(ct + 1) * P,
                                           mg * 512:(mg + 1) * 512])
                        wts.append(w1t)
                    pg = [ps() for _ in range(4)]
                    for ci, ct in enumerate(order):
                        for ml in range(4):
                            nc.tensor.matmul(
                                pg[ml], wts[ct][:, ml * P:(ml + 1) * P],
                                x1bf[ct],
                                start=(ci == 0), stop=False)
                    for ml in range(4):
                        mt = mg * 4 + ml
                        nc.tensor.matmul(
                            pg[ml], ns1g[0:1, ml * P:(ml + 1) * P], murow2,
                            start=False, stop=True)
                        zt = zp.tile([P, TOK], BF16, tag="zt", name="zt")
                        nc.vector.tensor_tensor(zt, pg[ml], rstd2_rep, ALU.mult)
                        nc.scalar.activation(gT[:, mt, :], zt, AF.Gelu_apprx_tanh)

                # matmul2 + residual -> outT
                for co in range(CT):
                    w2t = w2p.tile([P, MT, P], BF16, tag="w2t", name="w2t")
                    nc.scalar.dma_start(
                        w2t, w2r_d.ap()[co].rearrange("mo p c -> p mo c"))
                    po = ps()
                    for mt in range(MT):
                        nc.tensor.matmul(po, w2t[:, mt, :], gT[:, mt, :],
                                         start=(mt == 0), stop=(mt == MT - 1))
                    x1r = x1p.tile([P, TOK], F32, tag="x1o", name="x1o")
                    nc.scalar.dma_start(x1r, x1_spill[co * P:(co + 1) * P, :])
                    ot2 = outp.tile([P, TOK], F32, tag="ot2", name="ot2")
                    nc.vector.tensor_tensor(ot2, po, x1r, ALU.add)
                    nc.scalar.dma_start(out_d.ap()[co * P:(co + 1) * P, :], ot2)

    nc.compile()
    return nc


def _host_prep(x, w_qkv, w1, w2, ln_w):
    x = np.asarray(x, dtype=np.float32)
    w_qkv = np.asarray(w_qkv, dtype=np.float32)
    w1 = np.asarray(w1, dtype=np.float32)
    w2 = np.asarray(w2, dtype=np.float32)
    ln_w = np.asarray(ln_w, dtype=np.float32)

    xT = np.ascontiguousarray(x.transpose(0, 2, 1))            # [B, C, T]
    xT_bf = xT.astype(ml_dtypes.bfloat16)
    x_flat = x.reshape(B * T, C)

    Wq = (ln_w[:, None] * w_qkv[:, 0 * C:1 * C]) * SCALE
    Wk = ln_w[:, None] * w_qkv[:, 1 * C:2 * C]
    Wv = ln_w[:, None] * w_qkv[:, 2 * C:3 * C]
    nsq_full = -Wq.sum(0, dtype=np.float64).astype(np.float32)
    nsk_full = -Wk.sum(0, dtype=np.float64).astype(np.float32)
    nsv_full = -Wv.sum(0, dtype=np.float64).astype(np.float32)

    w1s = ln_w[:, None] * w1
    w1_bf = w1s.astype(ml_dtypes.bfloat16)
    ns1 = -w1s.sum(0, dtype=np.float64).astype(np.float32)
    # w2 reordered: [CT, MT, P(m), P(c)]
    w2r = np.ascontiguousarray(
        w2.reshape(MT, P, CT, P).transpose(2, 0, 1, 3)).astype(ml_dtypes.bfloat16)

    masks = np.zeros((4, P, 512), np.float32)
    for m in range(4):
        s_idx = np.arange(P)[:, None] + P * m
        t_idx = np.arange(512)[None, :]
        masks[m] = (t_idx >= s_idx).astype(np.float32)
    masks = masks.astype(ml_dtypes.bfloat16)

    in_maps = []
    for r in range(R):
        cs = slice(256 * r, 256 * (r + 1))
        b_own, tb_own = r // NT, r % NT
        in_maps.append({
            "xT": xT_bf,
            "x_own": np.ascontiguousarray(x_flat[TOK * r: TOK * (r + 1)]),
            "xT_own": np.ascontiguousarray(
                xT[b_own][:, 512 * tb_own: 512 * (tb_own + 1)]),
            "wq": np.ascontiguousarray(Wq[:, cs]).astype(ml_dtypes.bfloat16),
            "wk": np.ascontiguousarray(Wk[:, cs]).astype(ml_dtypes.bfloat16),
            "wv": np.ascontiguousarray(Wv[:, cs]).astype(ml_dtypes.bfloat16),
            "nsq": np.ascontiguousarray(nsq_full[cs]).astype(ml_dtypes.bfloat16),
            "nsk": np.ascontiguousarray(nsk_full[cs]).astype(ml_dtypes.bfloat16),
            "nsv": np.ascontiguousarray(nsv_full[cs]).astype(ml_dtypes.bfloat16),
            "w1": w1_bf,
            "ns1": ns1.astype(ml_dtypes.bfloat16),
            "w2r": w2r,
            "masks": masks,
        })
    return in_maps


def get_nc():
    if "nc" not in _CACHE:
        _CACHE["nc"] = _build()
    return _CACHE["nc"]


def run(in_maps, **kw):
    nc = get_nc()
    return run_bass_kernel_spmd(nc, in_maps, core_ids=list(range(R)), **kw)


def kernel(x, w_qkv, w1, w2, ln_w, **kw_unused):
    in_maps = _host_prep(x, w_qkv, w1, w2, ln_w)
    res = run(in_maps)
    out_flat = np.empty((B * T, C), np.float32)
    for r in range(R):
        out_flat[TOK * r: TOK * (r + 1)] = res.results[r]["outT"].T
    return out_flat.reshape(B, T, C)


# revision 17
# speedup vs baseline: 1.1849x; 1.0239x over previous
"""Trainium2 Bass kernel for nn_Block_10024453669245 (dense transformer block).

Strategy (8 NeuronCores):
  - warmup: dummy 32B AllGather prepays collective-communicator init.
  - Phase A: per-core LN1 stats on its 512 own tokens + tiny AllGather.
  - Phase B: QKV tensor-parallel over heads (2 heads/core). fp32r matmuls
    against host-transposed xT. LN1 is folded in: the rank-1 term
    (-colsum x murstd) is added via a K=1 fp32r matmul inside the same
    PSUM accumulation group; eviction is a single DVE mult by rstd.
    Produces qT,kT [d,t] and v [t,d] in bf16, resident in SBUF.
  - Phase C: causal attention head-major, no-max-sub softmax, S^T tiles,
    exp on ACT, causal masks on diagonal tiles, O^T and denominator
    accumulated on PE. Per-head AllToAll (2MB) fires as soon as that
    head's outputs are done, overlapping the other head's attention.
  - Phase E: MLP token-sharded (512 tokens/core) in bf16. ln2's weight is
    folded into w1 (host), the mean term via K=1 matmul fold, rstd2 at
    PSUM eviction. gelu = ACT Gelu_apprx_tanh. Residual from f32 spill.
    Output written transposed [C, 512] per core; host reassembles.
  DMAs are spread over sync/gpsimd/vector queues to avoid serializing.
"""
import sys, math

sys.path.insert(0, "/opt/trn_rl_repo")

import numpy as np
import ml_dtypes

import concourse.bass as bass
import concourse.tile as tile
from concourse import bacc, mybir
from concourse.bass_utils import run_bass_kernel_spmd

# ---------------- constants (hardcoded problem shape) ----------------
P = 128
B, T, C = 2, 2048, 2048
H, D = 16, 128
R = 8                 # cores
HL = H // R           # heads per core
TOK = B * T // R      # own tokens per core
CT = C // P           # 16 c-tiles
NT = T // 512         # 4 t-blocks per batch
M1 = 4 * C            # 8192
MT = M1 // P          # 64 m-tiles
MG = 16               # m-groups of 4 m-tiles (512 cols) for matmul1
EPS = 1e-5
SCALE = 1.0 / math.sqrt(D)

F32 = mybir.dt.float32
F32R = mybir.dt.float32r
BF16 = mybir.dt.bfloat16
AF = mybir.ActivationFunctionType
ALU = mybir.AluOpType

_CACHE = {}
DEBUG = False


def _pbc(t, n_free):
    """partition-broadcast AP over a 1-D dram tile view."""
    return bass.AP(tensor=t.tensor, offset=t.offset, ap=[[0, P], [1, n_free]])


def _row(ap1d):
    return ap1d.rearrange("(o t) -> o t", o=1)


def _build():
    nc = bacc.Bacc("TRN2", target_bir_lowering=False, debug=False, num_devices=R)

    # ---------------- I/O ----------------
    xT_d = nc.dram_tensor("xT", [B, C, T], BF16, kind="ExternalInput")
    xT_own_d = nc.dram_tensor("xT_own", [C, TOK], F32, kind="ExternalInput")
    xt_own_d = nc.dram_tensor("xt_own", [C, 512], BF16, kind="ExternalInput")
    wq_d = nc.dram_tensor("wq", [C, HL * D], BF16, kind="ExternalInput")
    wk_d = nc.dram_tensor("wk", [C, HL * D], BF16, kind="ExternalInput")
    wv_d = nc.dram_tensor("wv", [C, HL * D], BF16, kind="ExternalInput")
    nsq_d = nc.dram_tensor("nsq", [HL * D], BF16, kind="ExternalInput")
    nsk_d = nc.dram_tensor("nsk", [HL * D], BF16, kind="ExternalInput")
    nsv_d = nc.dram_tensor("nsv", [HL * D], BF16, kind="ExternalInput")
    w1_d = nc.dram_tensor("w1", [C, M1], BF16, kind="ExternalInput")
    ns1_d = nc.dram_tensor("ns1", [M1], BF16, kind="ExternalInput")
    w2r_d = nc.dram_tensor("w2r", [CT, MT, P, P], BF16, kind="ExternalInput")
    masks_d = nc.dram_tensor("masks", [4, P, 512], BF16, kind="ExternalInput")
    out_d = nc.dram_tensor("outT", [C, TOK], F32, kind="ExternalOutput")

    with tile.TileContext(nc) as tc:
        with tc.tile_pool(name="dram", bufs=1, space="DRAM") as dram, \
             tc.tile_pool(name="psum", bufs=8, space="PSUM") as psum, \
             tc.tile_pool(name="singles", bufs=1) as singles:

            # internal DRAM
            warm_in = dram.tile([8], F32)
            warm_out = dram.tile([R, 8], F32)
            stats_loc = dram.tile([2, TOK], F32)
            stats_g = dram.tile([R, 2, TOK], F32)
            a2a_in = [dram.tile([R, P, 512], F32, name=f"a2a_in{h}")
                      for h in range(HL)]
            a2a_out = [dram.tile([R, P, 512], F32, name=f"a2a_out{h}")
                       for h in range(HL)]
            x1_spill = dram.tile([C, TOK], F32)
            mlp_stat_b = dram.tile([2, TOK], F32)

            def ps():
                return psum.tile([P, 512], F32, tag="ps", name="ps")

            # warmup collective: pays communicator init while phase A runs
            nc.gpsimd.collective_compute(
                "AllGather", ALU.bypass, replica_groups=[list(range(R))],
                ins=[warm_in.opt()], outs=[warm_out.opt()])

            # small constants
            eps_t = singles.tile([P, 1], F32)
            nc.vector.memset(eps_t, EPS)
            ones_bf = singles.tile([P, 1], BF16)
            nc.vector.memset(ones_bf, 1.0)
            ones_f32 = singles.tile([P, 1], F32)
            nc.vector.memset(ones_f32, 1.0)

            # =========== Phase B+C pools (opened early: weight DMAs
            # go out on three parallel queues before phase A traffic) =======
            _wpool_cm = tc.tile_pool(name="wqkv", bufs=1)
            wpool = _wpool_cm.__enter__()
            wq_t = wpool.tile([P, CT, HL * D], BF16)
            wk_t = wpool.tile([P, CT, HL * D], BF16)
            wv_t = wpool.tile([P, CT, HL * D], BF16)
            nsq_t = wpool.tile([1, HL * D], BF16)
            nc.gpsimd.dma_start(nsq_t, _row(nsq_d.ap()))
            nsk_t = wpool.tile([1, HL * D], BF16)
            nc.gpsimd.dma_start(nsk_t, _row(nsk_d.ap()))
            nsv_t = wpool.tile([1, HL * D], BF16)
            nc.gpsimd.dma_start(nsv_t, _row(nsv_d.ap()))

            # =========== Phase A: LN1 stats on own tokens (from xt_own via
            # PE ones-matmuls), interleaved with per-ko weight loads ========
            with tc.tile_pool(name="stA", bufs=1) as stA:
                pmu0 = ps()
                psq0 = ps()
                wqr = wq_d.ap().rearrange("(ko p) n -> p ko n", p=P)
                wkr = wk_d.ap().rearrange("(ko p) n -> p ko n", p=P)
                wvr = wv_d.ap().rearrange("(ko p) n -> p ko n", p=P)
                for ko in range(CT):
                    xo = stA.tile([P, 512], BF16, tag="xo", name="xo", bufs=6)
                    nc.sync.dma_start(xo, xt_own_d.ap()[ko * P:(ko + 1) * P, :])
                    nc.sync.dma_start(wq_t[:, ko], wqr[:, ko])
                    nc.sync.dma_start(wv_t[:, ko], wvr[:, ko])
                    nc.scalar.dma_start(wk_t[:, ko], wkr[:, ko])
                    sqx = stA.tile([P, 512], BF16, tag="sqx", name="sqx", bufs=4)
                    nc.vector.tensor_tensor(sqx, xo, xo, ALU.mult)
                    nc.tensor.matmul(pmu0[0:1, :], ones_bf, xo,
                                     start=(ko == 0), stop=(ko == CT - 1))
                    nc.tensor.matmul(psq0[0:1, :], ones_bf, sqx,
                                     start=(ko == 0), stop=(ko == CT - 1))
                muA = stA.tile([1, 512], F32, tag="muA", name="muA")
                nc.vector.tensor_scalar(muA, pmu0[0:1, :], 1.0 / C, None, ALU.mult)
                varA = stA.tile([1, 512], F32, tag="varA", name="varA")
                nc.vector.tensor_scalar(varA, psq0[0:1, :], 1.0 / C, None, ALU.mult)
                musqA = stA.tile([1, 512], F32, tag="musqA", name="musqA")
                nc.vector.tensor_tensor(musqA, muA, muA, ALU.mult)
                nc.vector.tensor_tensor(varA, varA, musqA, ALU.subtract)
                rstdA = stA.tile([1, 512], F32, tag="rstdA", name="rstdA")
                nc.scalar.activation(rstdA, varA, AF.Sqrt, bias=eps_t[0:1])
                nc.vector.reciprocal_approx_fast(out=rstdA, in_=rstdA)
                murstdA = stA.tile([1, 512], F32, tag="murstdA", name="murstdA")
                nc.vector.tensor_tensor(murstdA, muA, rstdA, ALU.mult)
                nc.gpsimd.dma_start(_row(stats_loc[0, :]), rstdA)
                nc.gpsimd.dma_start(_row(stats_loc[1, :]), murstdA)
            nc.gpsimd.collective_compute(
                "AllGather", ALU.bypass,
                replica_groups=[list(range(R))],
                ins=[stats_loc.opt()], outs=[stats_g.opt()])
            masks_t = wpool.tile([P, 4, 512], BF16)
            nc.scalar.dma_start(
                masks_t, masks_d.ap().rearrange("m p t -> p m t"))

            # =========== Phase B+C pools ===========
            with tc.tile_pool(name="qkvres", bufs=1) as qkvres, \
                 tc.tile_pool(name="xtp", bufs=8) as xtp, \
                 tc.tile_pool(name="reps", bufs=4) as reps, \
                 tc.tile_pool(name="tmps", bufs=4) as tmps, \
                 tc.tile_pool(name="attn", bufs=3) as attnp:

                # persistent qkv (bf16)
                qT = [[qkvres.tile([P, T], BF16, name=f"qT{h}{b}")
                       for b in range(B)] for h in range(HL)]
                kT = [[qkvres.tile([P, T], BF16, name=f"kT{h}{b}")
                       for b in range(B)] for h in range(HL)]
                vsb = [qkvres.tile([P, T // P, HL * D], BF16, name=f"v{b}")
                       for b in range(B)]

                # =========== Phase B: QKV ===========
                for b in range(B):
                    for tb in range(NT):
                        j = NT * b + tb
                        t0 = 512 * tb
                        murow_f = reps.tile([1, 512], F32, tag="murowf", name="murow_f")
                        nc.gpsimd.dma_start(murow_f, _row(stats_g[j, 1, :]))
                        murow = reps.tile([1, 512], BF16, tag="murow", name="murow")
                        nc.vector.tensor_copy(murow, murow_f)
                        rstd_rep = reps.tile([P, 512], F32, tag="rrep", name="rstd_rep")
                        nc.gpsimd.dma_start(rstd_rep, _pbc(stats_g[j, 0, :], 512))

                        pq = [ps() for _ in range(HL)]
                        pk = [ps() for _ in range(HL)]
                        # one bank per 128-token v subtile (start=True clears
                        # the whole bank, chains must not share one)
                        pv = [ps() for _ in range(4)]
                        for ko in range(CT):
                            xt = xtp.tile([P, 512], BF16, tag="xt", name="xt")
                            nc.sync.dma_start(
                                xt,
                                xT_d.ap()[b, ko * P:(ko + 1) * P, t0:t0 + 512])
                            st_flag = ko == 0
                            for hl in range(HL):
                                nc.tensor.matmul(
                                    pq[hl], wq_t[:, ko, hl * D:(hl + 1) * D], xt,
                                    start=st_flag, stop=False)
                                nc.tensor.matmul(
                                    pk[hl], wk_t[:, ko, hl * D:(hl + 1) * D], xt,
                                    start=st_flag, stop=False)
                            for ss in range(4):
                                nc.tensor.matmul(
                                    pv[ss][:, 0:256],
                                    xt[:, ss * P:(ss + 1) * P], wv_t[:, ko, :],
                                    start=st_flag, stop=False)
                        # rank-1 LN fold: += (-colsum) x murstd  (K=1 matmul)
                        for hl in range(HL):
                            nc.tensor.matmul(
                                pq[hl], nsq_t[0:1, hl * D:(hl + 1) * D], murow,
                                start=False, stop=True)
                            nc.tensor.matmul(
                                pk[hl], nsk_t[0:1, hl * D:(hl + 1) * D], murow,
                                start=False, stop=True)
                        for ss in range(4):
                            nc.tensor.matmul(
                                pv[ss][:, 0:256],
                                murow[0:1, ss * P:(ss + 1) * P], nsv_t,
                                start=False, stop=True)
                        # evictions: single mult by rstd
                        for hl in range(HL):
                            nc.vector.tensor_tensor(
                                qT[hl][b][:, t0:t0 + 512], pq[hl], rstd_rep, ALU.mult)
                            nc.vector.tensor_tensor(
                                kT[hl][b][:, t0:t0 + 512], pk[hl], rstd_rep, ALU.mult)
                        for ss in range(4):
                            si = tb * 4 + ss
                            rstd_c = tmps.tile([P, 1], F32, tag="rc", name="rc")
                            nc.gpsimd.dma_start(
                                rstd_c,
                                stats_g[j, 0, ss * P:(ss + 1) * P]
                                .rearrange("(p o) -> p o", o=1))
                            nc.vector.tensor_scalar(
                                vsb[b][:, si, :], pv[ss][:, 0:256], rstd_c, None,
                                ALU.mult)

                # =========== Phase C: attention (head-major) ===========
                for hl in range(HL):
                    for b in range(B):
                        for tb in range(NT):
                            t0 = 512 * tb
                            n_s = 4 * (tb + 1)
                            pot = ps()
                            pden = ps()
                            for si in range(n_s):
                                pS = ps()
                                nc.tensor.matmul(
                                    pS, kT[hl][b][:, si * P:(si + 1) * P],
                                    qT[hl][b][:, t0:t0 + 512],
                                    start=True, stop=True)
                                pt = attnp.tile([P, 512], BF16, tag="pt", name="pt")
                                nc.scalar.activation(pt, pS, AF.Exp)
                                m = si - (n_s - 4)
                                if m >= 0:
                                    nc.vector.tensor_tensor(
                                        pt, pt, masks_t[:, m, :], ALU.mult)
                                nc.tensor.matmul(
                                    pot, vsb[b][:, si, hl * D:(hl + 1) * D], pt,
                                    start=(si == 0), stop=(si == n_s - 1))
                                nc.tensor.matmul(
                                    pden[0:1, :], ones_bf, pt,
                                    start=(si == 0), stop=(si == n_s - 1))
                            # normalize: OT / den
                            den_r = attnp.tile([1, 512], F32, tag="dr", name="den_r")
                            nc.vector.reciprocal_approx_fast(out=den_r, in_=pden[0:1, :])
                            den_d = dram.tile([512], F32, tag="den_d", bufs=4,
                                              name="den_d")
                            nc.gpsimd.dma_start(_row(den_d), den_r)
                            den_rep = reps.tile([P, 512], F32, tag="denrep",
                                                name="den_rep")
                            nc.gpsimd.dma_start(den_rep, _pbc(den_d, 512))
                            ot = attnp.tile([P, 512], F32, tag="ot", name="ot")
                            nc.vector.tensor_tensor(ot, pot, den_rep, ALU.mult)
                            nc.gpsimd.dma_start(a2a_in[hl][NT * b + tb, :, :], ot)
                    # per-head AllToAll fires as soon as head hl is done
                    nc.gpsimd.collective_compute(
                        "AllToAll", ALU.bypass,
                        replica_groups=[list(range(R))],
                        ins=[a2a_in[hl].opt()], outs=[a2a_out[hl].opt()])

            _wpool_cm.__exit__(None, None, None)

            # =========== Phase E: MLP (token-sharded, bf16) ===========
            with tc.tile_pool(name="mlp_x1", bufs=3) as x1p, \
                 tc.tile_pool(name="mlp_sq", bufs=2) as sqp, \
                 tc.tile_pool(name="mlp_x1bf", bufs=1) as x1bfp, \
                 tc.tile_pool(name="mlp_g", bufs=1) as gp, \
                 tc.tile_pool(name="mlp_w1", bufs=22) as w1p, \
                 tc.tile_pool(name="mlp_w2", bufs=2) as w2p, \
                 tc.tile_pool(name="mlp_z", bufs=4) as zp, \
                 tc.tile_pool(name="mlp_out", bufs=2) as outp:

                x1bf = [x1bfp.tile([P, TOK], BF16, name=f"x1bf{i}")
                        for i in range(CT)]
                # pass 1: build x1 tiles (evens first: only need a2a head 0),
                # stats matmuls, bf16 copy, f32 spill
                pmu = ps()
                psq = ps()
                order = [2 * i for i in range(CT // 2)] + \
                        [2 * i + 1 for i in range(CT // 2)]
                for idx, ct in enumerate(order):
                    xo = x1p.tile([P, TOK], F32, tag="xo2", name="xo2")
                    nc.gpsimd.dma_start(xo, xT_own_d.ap()[ct * P:(ct + 1) * P, :])
                    at = x1p.tile([P, TOK], F32, tag="at", name="at")
                    nc.gpsimd.dma_start(at, a2a_out[ct % 2][ct // 2])
                    x1 = x1p.tile([P, TOK], F32, tag="x1", name="x1")
                    nc.vector.tensor_tensor(x1, xo, at, ALU.add)
                    nc.gpsimd.dma_start(x1_spill[ct * P:(ct + 1) * P, :], x1)
                    nc.vector.tensor_copy(x1bf[ct], x1)
                    sq2 = sqp.tile([P, TOK], BF16, tag="sq2", name="sq2")
                    nc.vector.tensor_tensor(sq2, x1bf[ct], x1bf[ct], ALU.mult)
                    nc.tensor.matmul(pmu[0:1, :], ones_bf, x1bf[ct],
                                     start=(idx == 0), stop=(idx == CT - 1))
                    nc.tensor.matmul(psq[0:1, :], ones_bf, sq2,
                                     start=(idx == 0), stop=(idx == CT - 1))
                # finalize stats: mu = pmu/C ; var = psq/C - mu^2
                mu2 = singles.tile([1, TOK], F32)
                nc.vector.tensor_scalar(mu2, pmu[0:1, :], 1.0 / C, None, ALU.mult)
                var2 = singles.tile([1, TOK], F32)
                nc.vector.tensor_scalar(var2, psq[0:1, :], 1.0 / C, None, ALU.mult)
                musq = singles.tile([1, TOK], F32)
                nc.vector.tensor_tensor(musq, mu2, mu2, ALU.mult)
                nc.vector.tensor_tensor(var2, var2, musq, ALU.subtract)
                rstd2 = singles.tile([1, TOK], F32)
                nc.scalar.activation(rstd2, var2, AF.Sqrt, bias=eps_t[0:1])
                nc.vector.reciprocal_approx_fast(out=rstd2, in_=rstd2)
                nc.gpsimd.dma_start(_row(mlp_stat_b[0, :]), mu2)
                nc.gpsimd.dma_start(_row(mlp_stat_b[1, :]), rstd2)
                murow2 = singles.tile([1, TOK], BF16)
                nc.vector.tensor_copy(murow2, mu2)
                rstd2_rep = singles.tile([P, TOK], F32)
                nc.gpsimd.dma_start(rstd2_rep, _pbc(mlp_stat_b[1, :], TOK))

                # matmul1 (+ mean fold) -> *rstd2 -> gelu -> gT
                gT = gp.tile([P, MT, TOK], BF16)
                evens = order[:CT // 2]
                odds = order[CT // 2:]
                for mgp in range(MG // 2):
                    mgs = (2 * mgp, 2 * mgp + 1)
                    ns1gs, wtss, pgs = {}, {}, {}
                    for mg in mgs:
                        ns1g = zp.tile([1, 512], BF16, tag="ns1g", name="ns1g")
                        nc.gpsimd.dma_start(
                            ns1g, _row(ns1_d.ap()[mg * 512:(mg + 1) * 512]))
                        ns1gs[mg] = ns1g
                        wts = []
                        for ct in range(CT):
                            w1t = w1p.tile([P, 512], BF16, tag="w1t", name="w1t")
                            nc.sync.dma_start(
                                w1t, w1_d.ap()[ct * P:(ct + 1) * P,
                                               mg * 512:(mg + 1) * 512])
                            wts.append(w1t)
                        wtss[mg] = wts
                        pgs[mg] = [ps() for _ in range(4)]
                    for group in (evens, odds):
                        for mg in mgs:
                            for ci, ct in enumerate(group):
                                st_f = group is evens and ci == 0
                                for ml in range(4):
                                    nc.tensor.matmul(
                                        pgs[mg][ml],
                                        wtss[mg][ct][:, ml * P:(ml + 1) * P],
                                        x1bf[ct],
                                        start=st_f, stop=False)
                    for mg in mgs:
                        for ml in range(4):
                            mt = mg * 4 + ml
                            nc.tensor.matmul(
                                pgs[mg][ml], ns1gs[mg][0:1, ml * P:(ml + 1) * P],
                                murow2, start=False, stop=True)
                            zt = zp.tile([P, TOK], BF16, tag="zt", name="zt")
                            nc.vector.tensor_tensor(zt, pgs[mg][ml], rstd2_rep,
                                                    ALU.mult)
                            nc.scalar.activation(gT[:, mt, :], zt,
                                                 AF.Gelu_apprx_tanh)

                # matmul2 + residual -> outT
                for co in range(CT):
                    w2t = w2p.tile([P, MT, P], BF16, tag="w2t", name="w2t")
                    nc.scalar.dma_start(
                        w2t, w2r_d.ap()[co].rearrange("mo p c -> p mo c"))
                    po = ps()
                    for mt in range(MT):
                        nc.tensor.matmul(po, w2t[:, mt, :], gT[:, mt, :],
                                         start=(mt == 0), stop=(mt == MT - 1))
                    x1r = x1p.tile([P, TOK], F32, tag="x1o", name="x1o")
                    nc.scalar.dma_start(x1r, x1_spill[co * P:(co + 1) * P, :])
                    ot2 = outp.tile([P, TOK], F32, tag="ot2", name="ot2")
                    nc.vector.tensor_tensor(ot2, po, x1r, ALU.add)
                    nc.scalar.dma_start(out_d.ap()[co * P:(co + 1) * P, :], ot2)

    nc.compile()
    return nc


def _host_prep(x, w_qkv, w1, w2, ln_w):
    x = np.asarray(x, dtype=np.float32)
    w_qkv = np.asarray(w_qkv, dtype=np.float32)
    w1 = np.asarray(w1, dtype=np.float32)
    w2 = np.asarray(w2, dtype=np.float32)
    ln_w = np.asarray(ln_w, dtype=np.float32)

    xT = np.ascontiguousarray(x.transpose(0, 2, 1))            # [B, C, T]
    xT_bf = xT.astype(ml_dtypes.bfloat16)
    x_flat = x.reshape(B * T, C)

    Wq = (ln_w[:, None] * w_qkv[:, 0 * C:1 * C]) * SCALE
    Wk = ln_w[:, None] * w_qkv[:, 1 * C:2 * C]
    Wv = ln_w[:, None] * w_qkv[:, 2 * C:3 * C]
    nsq_full = -Wq.sum(0, dtype=np.float64).astype(np.float32)
    nsk_full = -Wk.sum(0, dtype=np.float64).astype(np.float32)
    nsv_full = -Wv.sum(0, dtype=np.float64).astype(np.float32)

    w1s = ln_w[:, None] * w1
    w1_bf = w1s.astype(ml_dtypes.bfloat16)
    ns1 = -w1s.sum(0, dtype=np.float64).astype(np.float32)
    # w2 reordered: [CT, MT, P(m), P(c)]
    w2r = np.ascontiguousarray(
        w2.reshape(MT, P, CT, P).transpose(2, 0, 1, 3)).astype(ml_dtypes.bfloat16)

    masks = np.zeros((4, P, 512), np.float32)
    for m in range(4):
        s_idx = np.arange(P)[:, None] + P * m
        t_idx = np.arange(512)[None, :]
        masks[m] = (t_idx >= s_idx).astype(np.float32)
    masks = masks.astype(ml_dtypes.bfloat16)

    in_maps = []
    for r in range(R):
        cs = slice(256 * r, 256 * (r + 1))
        b_own, tb_own = r // NT, r % NT
        in_maps.append({
            "xT": xT_bf,
            "xT_own": np.ascontiguousarray(
                xT[b_own][:, 512 * tb_own: 512 * (tb_own + 1)]),
            "xt_own": np.ascontiguousarray(
                xT_bf[b_own][:, 512 * tb_own: 512 * (tb_own + 1)]),
            "wq": np.ascontiguousarray(Wq[:, cs]).astype(ml_dtypes.bfloat16),
            "wk": np.ascontiguousarray(Wk[:, cs]).astype(ml_dtypes.bfloat16),
            "wv": np.ascontiguousarray(Wv[:, cs]).astype(ml_dtypes.bfloat16),
            "nsq": np.ascontiguousarray(nsq_full[cs]).astype(ml_dtypes.bfloat16),
            "nsk": np.ascontiguousarray(nsk_full[cs]).astype(ml_dtypes.bfloat16),
            "nsv": np.ascontiguousarray(nsv_full[cs]).astype(ml_dtypes.bfloat16),
            "w1": w1_bf,
            "ns1": ns1.astype(ml_dtypes.bfloat16),
            "w2r": w2r,
            "masks": masks,
        })
    return in_maps


def get_nc():
    if "nc" not in _CACHE:
        _CACHE["nc"] = _build()
    return _CACHE["nc"]


def run(in_maps, **kw):
    nc = get_nc()
    return run_bass_kernel_spmd(nc, in_maps, core_ids=list(range(R)), **kw)


def kernel(x, w_qkv, w1, w2, ln_w, **kw_unused):
    in_maps = _host_prep(x, w_qkv, w1, w2, ln_w)
    res = run(in_maps)
    out_flat = np.empty((B * T, C), np.float32)
    for r in range(R):
        out_flat[TOK * r: TOK * (r + 1)] = res.results[r]["outT"].T
    return out_flat.reshape(B, T, C)


# revision 18
# speedup vs baseline: 1.2091x; 1.0204x over previous
"""Trainium2 Bass kernel for nn_Block_10024453669245 (dense transformer block).

Strategy (8 NeuronCores):
  - warmup: dummy 32B AllGather prepays collective-communicator init.
  - Phase A: per-core LN1 stats on its 512 own tokens + tiny AllGather.
  - Phase B: QKV tensor-parallel over heads (2 heads/core). fp32r matmuls
    against host-transposed xT. LN1 is folded in: the rank-1 term
    (-colsum x murstd) is added via a K=1 fp32r matmul inside the same
    PSUM accumulation group; eviction is a single DVE mult by rstd.
    Produces qT,kT [d,t] and v [t,d] in bf16, resident in SBUF.
  - Phase C: causal attention head-major, no-max-sub softmax, S^T tiles,
    exp on ACT, causal masks on diagonal tiles, O^T and denominator
    accumulated on PE. Per-head AllToAll (2MB) fires as soon as that
    head's outputs are done, overlapping the other head's attention.
  - Phase E: MLP token-sharded (512 tokens/core) in bf16. ln2's weight is
    folded into w1 (host), the mean term via K=1 matmul fold, rstd2 at
    PSUM eviction. gelu = ACT Gelu_apprx_tanh. Residual from f32 spill.
    Output written transposed [C, 512] per core; host reassembles.
  DMAs are spread over sync/gpsimd/vector queues to avoid serializing.
"""
import sys, math

sys.path.insert(0, "/opt/trn_rl_repo")

import numpy as np
import ml_dtypes

import concourse.bass as bass
import concourse.tile as tile
from concourse import bacc, mybir
from concourse.bass_utils import run_bass_kernel_spmd

# ---------------- constants (hardcoded problem shape) ----------------
P = 128
B, T, C = 2, 2048, 2048
H, D = 16, 128
R = 8                 # cores
HL = H // R           # heads per core
TOK = B * T // R      # own tokens per core
CT = C // P           # 16 c-tiles
NT = T // 512         # 4 t-blocks per batch
M1 = 4 * C            # 8192
MT = M1 // P          # 64 m-tiles
MG = 16               # m-groups of 4 m-tiles (512 cols) for matmul1
EPS = 1e-5
SCALE = 1.0 / math.sqrt(D)

F32 = mybir.dt.float32
F32R = mybir.dt.float32r
BF16 = mybir.dt.bfloat16
AF = mybir.ActivationFunctionType
ALU = mybir.AluOpType

_CACHE = {}
DEBUG = False


def _pbc(t, n_free):
    """partition-broadcast AP over a 1-D dram tile view."""
    return bass.AP(tensor=t.tensor, offset=t.offset, ap=[[0, P], [1, n_free]])


def _row(ap1d):
    return ap1d.rearrange("(o t) -> o t", o=1)


def _build():
    nc = bacc.Bacc("TRN2", target_bir_lowering=False, debug=False, num_devices=R)

    # ---------------- I/O ----------------
    xT_d = nc.dram_tensor("xT", [B, C, T], BF16, kind="ExternalInput")
    xT_own_d = nc.dram_tensor("xT_own", [C, TOK], F32, kind="ExternalInput")
    xt_own_d = nc.dram_tensor("xt_own", [C, 512], BF16, kind="ExternalInput")
    wq_d = nc.dram_tensor("wq", [C, HL * D], BF16, kind="ExternalInput")
    wk_d = nc.dram_tensor("wk", [C, HL * D], BF16, kind="ExternalInput")
    wv_d = nc.dram_tensor("wv", [C, HL * D], BF16, kind="ExternalInput")
    nsq_d = nc.dram_tensor("nsq", [HL * D], BF16, kind="ExternalInput")
    nsk_d = nc.dram_tensor("nsk", [HL * D], BF16, kind="ExternalInput")
    nsv_d = nc.dram_tensor("nsv", [HL * D], BF16, kind="ExternalInput")
    w1_d = nc.dram_tensor("w1", [C, M1], BF16, kind="ExternalInput")
    ns1_d = nc.dram_tensor("ns1", [M1], BF16, kind="ExternalInput")
    w2r_d = nc.dram_tensor("w2r", [CT, MT, P, P], BF16, kind="ExternalInput")
    masks_d = nc.dram_tensor("masks", [P, P], BF16, kind="ExternalInput")
    out_d = nc.dram_tensor("outT", [C, TOK], F32, kind="ExternalOutput")

    with tile.TileContext(nc) as tc:
        with tc.tile_pool(name="dram", bufs=1, space="DRAM") as dram, \
             tc.tile_pool(name="psum", bufs=8, space="PSUM") as psum, \
             tc.tile_pool(name="singles", bufs=1) as singles:

            # internal DRAM
            warm_in = dram.tile([8], F32)
            warm_out = dram.tile([R, 8], F32)
            stats_loc = dram.tile([2, TOK], F32)
            stats_g = dram.tile([R, 2, TOK], F32)
            a2a_in = [dram.tile([R, P, 512], F32, name=f"a2a_in{h}")
                      for h in range(HL)]
            a2a_out = [dram.tile([R, P, 512], F32, name=f"a2a_out{h}")
                       for h in range(HL)]
            x1_spill = dram.tile([C, TOK], F32)
            mlp_stat_b = dram.tile([2, TOK], F32)

            def ps():
                return psum.tile([P, 512], F32, tag="ps", name="ps")

            # warmup collective: pays communicator init while phase A runs
            nc.gpsimd.collective_compute(
                "AllGather", ALU.bypass, replica_groups=[list(range(R))],
                ins=[warm_in.opt()], outs=[warm_out.opt()])

            # small constants
            eps_t = singles.tile([P, 1], F32)
            nc.vector.memset(eps_t, EPS)
            ones_bf = singles.tile([P, 1], BF16)
            nc.vector.memset(ones_bf, 1.0)
            ones_f32 = singles.tile([P, 1], F32)
            nc.vector.memset(ones_f32, 1.0)

            # =========== Phase B+C pools (opened early: weight DMAs
            # go out on three parallel queues before phase A traffic) =======
            _wpool_cm = tc.tile_pool(name="wqkv", bufs=1)
            wpool = _wpool_cm.__enter__()
            wq_t = wpool.tile([P, CT, HL * D], BF16)
            wk_t = wpool.tile([P, CT, HL * D], BF16)
            wv_t = wpool.tile([P, CT, HL * D], BF16)
            nsq_t = wpool.tile([1, HL * D], BF16)
            nc.gpsimd.dma_start(nsq_t, _row(nsq_d.ap()))
            nsk_t = wpool.tile([1, HL * D], BF16)
            nc.gpsimd.dma_start(nsk_t, _row(nsk_d.ap()))
            nsv_t = wpool.tile([1, HL * D], BF16)
            nc.gpsimd.dma_start(nsv_t, _row(nsv_d.ap()))

            # =========== Phase A: LN1 stats on own tokens (from xt_own via
            # PE ones-matmuls), interleaved with per-ko weight loads ========
            with tc.tile_pool(name="stA", bufs=1) as stA:
                pmu0 = ps()
                psq0 = ps()
                wqr = wq_d.ap().rearrange("(ko p) n -> p ko n", p=P)
                wkr = wk_d.ap().rearrange("(ko p) n -> p ko n", p=P)
                wvr = wv_d.ap().rearrange("(ko p) n -> p ko n", p=P)
                for ko in range(CT):
                    xo = stA.tile([P, 512], BF16, tag="xo", name="xo", bufs=6)
                    nc.sync.dma_start(xo, xt_own_d.ap()[ko * P:(ko + 1) * P, :])
                    nc.sync.dma_start(wq_t[:, ko], wqr[:, ko])
                    nc.sync.dma_start(wv_t[:, ko], wvr[:, ko])
                    nc.scalar.dma_start(wk_t[:, ko], wkr[:, ko])
                    sqx = stA.tile([P, 512], BF16, tag="sqx", name="sqx", bufs=4)
                    nc.vector.tensor_tensor(sqx, xo, xo, ALU.mult)
                    nc.tensor.matmul(pmu0[0:1, :], ones_bf, xo,
                                     start=(ko == 0), stop=(ko == CT - 1))
                    nc.tensor.matmul(psq0[0:1, :], ones_bf, sqx,
                                     start=(ko == 0), stop=(ko == CT - 1))
                muA = stA.tile([1, 512], F32, tag="muA", name="muA")
                nc.vector.tensor_scalar(muA, pmu0[0:1, :], 1.0 / C, None, ALU.mult)
                varA = stA.tile([1, 512], F32, tag="varA", name="varA")
                nc.vector.tensor_scalar(varA, psq0[0:1, :], 1.0 / C, None, ALU.mult)
                musqA = stA.tile([1, 512], F32, tag="musqA", name="musqA")
                nc.vector.tensor_tensor(musqA, muA, muA, ALU.mult)
                nc.vector.tensor_tensor(varA, varA, musqA, ALU.subtract)
                rstdA = stA.tile([1, 512], F32, tag="rstdA", name="rstdA")
                nc.scalar.activation(rstdA, varA, AF.Sqrt, bias=eps_t[0:1])
                nc.vector.reciprocal_approx_fast(out=rstdA, in_=rstdA)
                murstdA = stA.tile([1, 512], F32, tag="murstdA", name="murstdA")
                nc.vector.tensor_tensor(murstdA, muA, rstdA, ALU.mult)
                nc.gpsimd.dma_start(_row(stats_loc[0, :]), rstdA)
                nc.gpsimd.dma_start(_row(stats_loc[1, :]), murstdA)
            nc.gpsimd.collective_compute(
                "AllGather", ALU.bypass,
                replica_groups=[list(range(R))],
                ins=[stats_loc.opt()], outs=[stats_g.opt()])
            masks_t = wpool.tile([P, P], BF16)
            nc.scalar.dma_start(masks_t, masks_d.ap())

            # =========== Phase B+C pools ===========
            with tc.tile_pool(name="qkvres", bufs=1) as qkvres, \
                 tc.tile_pool(name="xtp", bufs=8) as xtp, \
                 tc.tile_pool(name="reps", bufs=4) as reps, \
                 tc.tile_pool(name="tmps", bufs=4) as tmps, \
                 tc.tile_pool(name="attn", bufs=3) as attnp:

                # persistent qkv (bf16)
                qT = [[qkvres.tile([P, T], BF16, name=f"qT{h}{b}")
                       for b in range(B)] for h in range(HL)]
                kT = [[qkvres.tile([P, T], BF16, name=f"kT{h}{b}")
                       for b in range(B)] for h in range(HL)]
                vsb = [qkvres.tile([P, T // P, HL * D], BF16, name=f"v{b}")
                       for b in range(B)]

                # =========== Phase B: QKV ===========
                for b in range(B):
                    for tb in range(NT):
                        j = NT * b + tb
                        t0 = 512 * tb
                        murow_f = reps.tile([1, 512], F32, tag="murowf", name="murow_f")
                        nc.gpsimd.dma_start(murow_f, _row(stats_g[j, 1, :]))
                        murow = reps.tile([1, 512], BF16, tag="murow", name="murow")
                        nc.vector.tensor_copy(murow, murow_f)
                        rstd_rep = reps.tile([P, 512], F32, tag="rrep", name="rstd_rep")
                        nc.gpsimd.dma_start(rstd_rep, _pbc(stats_g[j, 0, :], 512))

                        pq = [ps() for _ in range(HL)]
                        pk = [ps() for _ in range(HL)]
                        # one bank per 128-token v subtile (start=True clears
                        # the whole bank, chains must not share one)
                        pv = [ps() for _ in range(4)]
                        for ko in range(CT):
                            xt = xtp.tile([P, 512], BF16, tag="xt", name="xt")
                            nc.sync.dma_start(
                                xt,
                                xT_d.ap()[b, ko * P:(ko + 1) * P, t0:t0 + 512])
                            st_flag = ko == 0
                            for hl in range(HL):
                                nc.tensor.matmul(
                                    pq[hl], wq_t[:, ko, hl * D:(hl + 1) * D], xt,
                                    start=st_flag, stop=False)
                                nc.tensor.matmul(
                                    pk[hl], wk_t[:, ko, hl * D:(hl + 1) * D], xt,
                                    start=st_flag, stop=False)
                            for ss in range(4):
                                nc.tensor.matmul(
                                    pv[ss][:, 0:256],
                                    xt[:, ss * P:(ss + 1) * P], wv_t[:, ko, :],
                                    start=st_flag, stop=False)
                        # rank-1 LN fold: += (-colsum) x murstd  (K=1 matmul)
                        for hl in range(HL):
                            nc.tensor.matmul(
                                pq[hl], nsq_t[0:1, hl * D:(hl + 1) * D], murow,
                                start=False, stop=True)
                            nc.tensor.matmul(
                                pk[hl], nsk_t[0:1, hl * D:(hl + 1) * D], murow,
                                start=False, stop=True)
                        for ss in range(4):
                            nc.tensor.matmul(
                                pv[ss][:, 0:256],
                                murow[0:1, ss * P:(ss + 1) * P], nsv_t,
                                start=False, stop=True)
                        # evictions: single mult by rstd
                        for hl in range(HL):
                            nc.vector.tensor_tensor(
                                qT[hl][b][:, t0:t0 + 512], pq[hl], rstd_rep, ALU.mult)
                            nc.vector.tensor_tensor(
                                kT[hl][b][:, t0:t0 + 512], pk[hl], rstd_rep, ALU.mult)
                        for ss in range(4):
                            si = tb * 4 + ss
                            rstd_c = tmps.tile([P, 1], F32, tag="rc", name="rc")
                            nc.gpsimd.dma_start(
                                rstd_c,
                                stats_g[j, 0, ss * P:(ss + 1) * P]
                                .rearrange("(p o) -> p o", o=1))
                            nc.vector.tensor_scalar(
                                vsb[b][:, si, :], pv[ss][:, 0:256], rstd_c, None,
                                ALU.mult)

                # =========== Phase C: attention (head-major) ===========
                for hl in range(HL):
                    for b in range(B):
                        for tb in range(NT):
                            t0 = 512 * tb
                            n_s = 4 * (tb + 1)
                            pot = ps()
                            pden = ps()
                            for si in range(n_s):
                                m = si - (n_s - 4)
                                w0 = max(m, 0) * P   # masked-out prefix width
                                pS = ps()
                                nc.tensor.matmul(
                                    pS[:, w0:512],
                                    kT[hl][b][:, si * P:(si + 1) * P],
                                    qT[hl][b][:, t0 + w0:t0 + 512],
                                    start=True, stop=True)
                                pt = attnp.tile([P, 512], BF16, tag="pt", name="pt")
                                nc.scalar.activation(pt[:, w0:512], pS[:, w0:512],
                                                     AF.Exp)
                                if m >= 0:
                                    nc.vector.tensor_tensor(
                                        pt[:, w0:w0 + P], pt[:, w0:w0 + P],
                                        masks_t, ALU.mult)
                                nc.tensor.matmul(
                                    pot[:, w0:512],
                                    vsb[b][:, si, hl * D:(hl + 1) * D],
                                    pt[:, w0:512],
                                    start=(si == 0), stop=(si == n_s - 1))
                                nc.tensor.matmul(
                                    pden[0:1, w0:512], ones_bf, pt[:, w0:512],
                                    start=(si == 0), stop=(si == n_s - 1))
                            # normalize: OT / den
                            den_r = attnp.tile([1, 512], F32, tag="dr", name="den_r")
                            nc.vector.reciprocal_approx_fast(out=den_r, in_=pden[0:1, :])
                            den_d = dram.tile([512], F32, tag="den_d", bufs=4,
                                              name="den_d")
                            nc.gpsimd.dma_start(_row(den_d), den_r)
                            den_rep = reps.tile([P, 512], F32, tag="denrep",
                                                name="den_rep")
                            nc.gpsimd.dma_start(den_rep, _pbc(den_d, 512))
                            ot = attnp.tile([P, 512], F32, tag="ot", name="ot")
                            nc.vector.tensor_tensor(ot, pot, den_rep, ALU.mult)
                            nc.gpsimd.dma_start(a2a_in[hl][NT * b + tb, :, :], ot)
                    # per-head AllToAll fires as soon as head hl is done
                    nc.gpsimd.collective_compute(
                        "AllToAll", ALU.bypass,
                        replica_groups=[list(range(R))],
                        ins=[a2a_in[hl].opt()], outs=[a2a_out[hl].opt()])

            _wpool_cm.__exit__(None, None, None)

            # =========== Phase E: MLP (token-sharded, bf16) ===========
            with tc.tile_pool(name="mlp_x1", bufs=3) as x1p, \
                 tc.tile_pool(name="mlp_sq", bufs=2) as sqp, \
                 tc.tile_pool(name="mlp_x1bf", bufs=1) as x1bfp, \
                 tc.tile_pool(name="mlp_g", bufs=1) as gp, \
                 tc.tile_pool(name="mlp_w1", bufs=22) as w1p, \
                 tc.tile_pool(name="mlp_w2", bufs=2) as w2p, \
                 tc.tile_pool(name="mlp_z", bufs=4) as zp, \
                 tc.tile_pool(name="mlp_out", bufs=2) as outp:

                x1bf = [x1bfp.tile([P, TOK], BF16, name=f"x1bf{i}")
                        for i in range(CT)]
                # pass 1: build x1 tiles (evens first: only need a2a head 0),
                # stats matmuls, bf16 copy, f32 spill
                pmu = ps()
                psq = ps()
                order = [2 * i for i in range(CT // 2)] + \
                        [2 * i + 1 for i in range(CT // 2)]
                for idx, ct in enumerate(order):
                    xo = x1p.tile([P, TOK], F32, tag="xo2", name="xo2")
                    nc.scalar.dma_start(xo, xT_own_d.ap()[ct * P:(ct + 1) * P, :])
                    at = x1p.tile([P, TOK], F32, tag="at", name="at")
                    nc.gpsimd.dma_start(at, a2a_out[ct % 2][ct // 2])
                    x1 = x1p.tile([P, TOK], F32, tag="x1", name="x1")
                    nc.vector.tensor_tensor(x1, xo, at, ALU.add)
                    nc.gpsimd.dma_start(x1_spill[ct * P:(ct + 1) * P, :], x1)
                    nc.vector.tensor_copy(x1bf[ct], x1)
                    sq2 = sqp.tile([P, TOK], BF16, tag="sq2", name="sq2")
                    nc.vector.tensor_tensor(sq2, x1bf[ct], x1bf[ct], ALU.mult)
                    nc.tensor.matmul(pmu[0:1, :], ones_bf, x1bf[ct],
                                     start=(idx == 0), stop=(idx == CT - 1))
                    nc.tensor.matmul(psq[0:1, :], ones_bf, sq2,
                                     start=(idx == 0), stop=(idx == CT - 1))
                # finalize stats: mu = pmu/C ; var = psq/C - mu^2
                mu2 = singles.tile([1, TOK], F32)
                nc.vector.tensor_scalar(mu2, pmu[0:1, :], 1.0 / C, None, ALU.mult)
                var2 = singles.tile([1, TOK], F32)
                nc.vector.tensor_scalar(var2, psq[0:1, :], 1.0 / C, None, ALU.mult)
                musq = singles.tile([1, TOK], F32)
                nc.vector.tensor_tensor(musq, mu2, mu2, ALU.mult)
                nc.vector.tensor_tensor(var2, var2, musq, ALU.subtract)
                rstd2 = singles.tile([1, TOK], F32)
                nc.scalar.activation(rstd2, var2, AF.Sqrt, bias=eps_t[0:1])
                nc.vector.reciprocal_approx_fast(out=rstd2, in_=rstd2)
                nc.gpsimd.dma_start(_row(mlp_stat_b[0, :]), mu2)
                nc.gpsimd.dma_start(_row(mlp_stat_b[1, :]), rstd2)
                murow2 = singles.tile([1, TOK], BF16)
                nc.vector.tensor_copy(murow2, mu2)
                rstd2_rep = singles.tile([P, TOK], F32)
                nc.gpsimd.dma_start(rstd2_rep, _pbc(mlp_stat_b[1, :], TOK))

                # matmul1 (+ mean fold) -> *rstd2 -> gelu -> gT
                gT = gp.tile([P, MT, TOK], BF16)
                evens = order[:CT // 2]
                odds = order[CT // 2:]
                for mgp in range(MG // 2):
                    mgs = (2 * mgp, 2 * mgp + 1)
                    ns1gs, wtss, pgs = {}, {}, {}
                    for mg in mgs:
                        ns1g = zp.tile([1, 512], BF16, tag="ns1g", name="ns1g")
                        nc.gpsimd.dma_start(
                            ns1g, _row(ns1_d.ap()[mg * 512:(mg + 1) * 512]))
                        ns1gs[mg] = ns1g
                        wts = []
                        for ct in range(CT):
                            w1t = w1p.tile([P, 512], BF16, tag="w1t", name="w1t")
                            nc.sync.dma_start(
                                w1t, w1_d.ap()[ct * P:(ct + 1) * P,
                                               mg * 512:(mg + 1) * 512])
                            wts.append(w1t)
                        wtss[mg] = wts
                        pgs[mg] = [ps() for _ in range(4)]
                    for group in (evens, odds):
                        for mg in mgs:
                            for ci, ct in enumerate(group):
                                st_f = group is evens and ci == 0
                                for ml in range(4):
                                    nc.tensor.matmul(
                                        pgs[mg][ml],
                                        wtss[mg][ct][:, ml * P:(ml + 1) * P],
                                        x1bf[ct],
                                        start=st_f, stop=False)
                    for mg in mgs:
                        for ml in range(4):
                            mt = mg * 4 + ml
                            nc.tensor.matmul(
                                pgs[mg][ml], ns1gs[mg][0:1, ml * P:(ml + 1) * P],
                                murow2, start=False, stop=True)
                            zt = zp.tile([P, TOK], BF16, tag="zt", name="zt")
                            nc.vector.tensor_tensor(zt, pgs[mg][ml], rstd2_rep,
                                                    ALU.mult)
                            nc.scalar.activation(gT[:, mt, :], zt,
                                                 AF.Gelu_apprx_tanh)

                # matmul2 + residual -> outT
                for co in range(CT):
                    w2t = w2p.tile([P, MT, P], BF16, tag="w2t", name="w2t")
                    nc.scalar.dma_start(
                        w2t, w2r_d.ap()[co].rearrange("mo p c -> p mo c"))
                    po = ps()
                    for mt in range(MT):
                        nc.tensor.matmul(po, w2t[:, mt, :], gT[:, mt, :],
                                         start=(mt == 0), stop=(mt == MT - 1))
                    x1r = x1p.tile([P, TOK], F32, tag="x1o", name="x1o")
                    nc.scalar.dma_start(x1r, x1_spill[co * P:(co + 1) * P, :])
                    ot2 = outp.tile([P, TOK], F32, tag="ot2", name="ot2")
                    nc.vector.tensor_tensor(ot2, po, x1r, ALU.add)
                    nc.scalar.dma_start(out_d.ap()[co * P:(co + 1) * P, :], ot2)

    nc.compile()
    return nc


def _host_prep(x, w_qkv, w1, w2, ln_w):
    x = np.asarray(x, dtype=np.float32)
    w_qkv = np.asarray(w_qkv, dtype=np.float32)
    w1 = np.asarray(w1, dtype=np.float32)
    w2 = np.asarray(w2, dtype=np.float32)
    ln_w = np.asarray(ln_w, dtype=np.float32)

    xT = np.ascontiguousarray(x.transpose(0, 2, 1))            # [B, C, T]
    xT_bf = xT.astype(ml_dtypes.bfloat16)
    x_flat = x.reshape(B * T, C)

    Wq = (ln_w[:, None] * w_qkv[:, 0 * C:1 * C]) * SCALE
    Wk = ln_w[:, None] * w_qkv[:, 1 * C:2 * C]
    Wv = ln_w[:, None] * w_qkv[:, 2 * C:3 * C]
    nsq_full = -Wq.sum(0, dtype=np.float64).astype(np.float32)
    nsk_full = -Wk.sum(0, dtype=np.float64).astype(np.float32)
    nsv_full = -Wv.sum(0, dtype=np.float64).astype(np.float32)

    w1s = ln_w[:, None] * w1
    w1_bf = w1s.astype(ml_dtypes.bfloat16)
    ns1 = -w1s.sum(0, dtype=np.float64).astype(np.float32)
    # w2 reordered: [CT, MT, P(m), P(c)]
    w2r = np.ascontiguousarray(
        w2.reshape(MT, P, CT, P).transpose(2, 0, 1, 3)).astype(ml_dtypes.bfloat16)

    masks = (np.arange(P)[None, :] >= np.arange(P)[:, None]).astype(
        np.float32).astype(ml_dtypes.bfloat16)

    in_maps = []
    for r in range(R):
        cs = slice(256 * r, 256 * (r + 1))
        b_own, tb_own = r // NT, r % NT
        in_maps.append({
            "xT": xT_bf,
            "xT_own": np.ascontiguousarray(
                xT[b_own][:, 512 * tb_own: 512 * (tb_own + 1)]),
            "xt_own": np.ascontiguousarray(
                xT_bf[b_own][:, 512 * tb_own: 512 * (tb_own + 1)]),
            "wq": np.ascontiguousarray(Wq[:, cs]).astype(ml_dtypes.bfloat16),
            "wk": np.ascontiguousarray(Wk[:, cs]).astype(ml_dtypes.bfloat16),
            "wv": np.ascontiguousarray(Wv[:, cs]).astype(ml_dtypes.bfloat16),
            "nsq": np.ascontiguousarray(nsq_full[cs]).astype(ml_dtypes.bfloat16),
            "nsk": np.ascontiguousarray(nsk_full[cs]).astype(ml_dtypes.bfloat16),
            "nsv": np.ascontiguousarray(nsv_full[cs]).astype(ml_dtypes.bfloat16),
            "w1": w1_bf,
            "ns1": ns1.astype(ml_dtypes.bfloat16),
            "w2r": w2r,
            "masks": masks,
        })
    return in_maps


def get_nc():
    if "nc" not in _CACHE:
        _CACHE["nc"] = _build()
    return _CACHE["nc"]


def run(in_maps, **kw):
    nc = get_nc()
    return run_bass_kernel_spmd(nc, in_maps, core_ids=list(range(R)), **kw)


def kernel(x, w_qkv, w1, w2, ln_w, **kw_unused):
    in_maps = _host_prep(x, w_qkv, w1, w2, ln_w)
    res = run(in_maps)
    out_flat = np.empty((B * T, C), np.float32)
    for r in range(R):
        out_flat[TOK * r: TOK * (r + 1)] = res.results[r]["outT"].T
    return out_flat.reshape(B, T, C)
